# revision 1
# baseline (speedup 1.0000x reference)
"""Trainium2 Bass kernel for the 4-layer PNA GNN (nn_Net_70025146794268).

Self-contained: accepts FULL inputs, shards graph-parallel over 8 NeuronCores,
runs a single SPMD Bass/Tile program, gathers the [G,1] output on host.

Per-core design (128 graphs, 16128 nodes, 57600 edges):
  * feature-major layouts throughout: tensors are [128 feat-partitions, nodes]
  * message m = Hp2[src] + Q[edge_attr] (pre_b folded into Q); the dst-side
    term Hp1[dst] is folded into the post matmul as extra K-blocks because
    mean/min/max shift by it and std is shift-invariant
  * per-node degree-padded slot tables (D in {1,2,4,8,16} regions) gathered via
    SBUF-source transpose dma_gather from the Hp2 token buffer; all four PNA
    stats computed by halving trees over the slot-major padded layout
  * BatchNorm is global over all 129024 nodes: per-core sums all-reduced via
    a tiny gpsimd collective each layer
  * pooling via per-graph 0/1 matmuls, then the readout MLP on-device in f32
"""

import time

import numpy as np
import ml_dtypes

import concourse.bacc as bacc
import concourse.tile as tile
import concourse.mybir as mybir
from concourse import bass2jax

HF = mybir.dt.float16
F32 = mybir.dt.float32
I16 = mybir.dt.int16

G, NPG, EPG, F = 1024, 126, 450, 126
NCORES = 8
GC = G // NCORES            # 128 graphs per core
NLOC = GC * NPG             # 16128 nodes per core
ELOC = GC * EPG             # 57600 edges per core
FP = 128                    # padded feature dim
EPS = 1e-5
NTOT = float(G * NPG)       # BN normalizer (all nodes, all cores)
SLOT_CAP = 2048             # max padded slots per window

LAST_HW_EXEC_NS = None


def _np2(d):
    p = 1
    while p < d:
        p *= 2
    return p


def _wrap_idx_table(flat):
    """[n] int16 -> [128, n//16] wrapped (i%16, i//16), replicated x8."""
    n = len(flat)
    assert n % 16 == 0
    tab = np.zeros((128, n // 16), np.int16)
    a = np.asarray(flat, np.int16).reshape(n // 16, 16).T  # [16, n//16]
    for g in range(8):
        tab[16 * g:16 * (g + 1)] = a
    return tab


def _prep(inputs):
    """Host-side preprocessing: common layout plan + per-core tables."""
    x = np.asarray(inputs["x"], np.float32)
    ei = np.asarray(inputs["edge_index"], np.int64)
    ea = np.asarray(inputs["edge_attr"], np.int64)
    dh = np.asarray(inputs["deg_hist"], np.float64)

    bins = np.arange(dh.shape[0], dtype=np.float64)
    avg_log = float((dh * np.log(bins + 1.0)).sum() / dh.sum())

    cores = []
    for c in range(NCORES):
        n0, e0 = c * NLOC, c * ELOC
        cores.append(dict(
            src=ei[0, e0:e0 + ELOC] - n0,
            dst=ei[1, e0:e0 + ELOC] - n0,
            at=ea[e0:e0 + ELOC],
        ))
        cores[-1]["deg"] = np.bincount(cores[-1]["dst"], minlength=NLOC)

    dmax = max(int(co["deg"].max()) for co in cores)
    if dmax > 32:
        raise RuntimeError(f"max degree {dmax} > 32 unsupported")
    REG_DS = [d for d in [32, 16, 8, 4, 2, 1] if d <= _np2(dmax)]

    # common padded region sizes (max over cores, 128-aligned)
    reg_counts = {D: 0 for D in REG_DS}
    for co in cores:
        np2s = np.array([_np2(max(int(d), 1)) for d in co["deg"]])
        for D in REG_DS:
            reg_counts[D] = max(reg_counts[D], int((np2s == D).sum()))
    npad = {}
    for D in REG_DS:
        n = -(-reg_counts[D] // 128) * 128
        if D == 1:
            n += 128          # guaranteed fake block (BN correction column)
        npad[D] = n

    windows = []              # (D, node_off, nw, slot_off)
    node_off, slot_off = 0, 0
    base = {}
    for D in REG_DS:
        base[D] = node_off
        win = max(128, min(512, SLOT_CAP // D))
        off = 0
        while off < npad[D]:
            nw = min(win, npad[D] - off)
            windows.append((D, node_off + off, nw, slot_off))
            slot_off += D * nw
            off += nw
        node_off += npad[D]
    W, TOT = node_off, slot_off
    n_fake = W - NLOC
    assert W % 128 == 0 and TOT % 128 == 0

    boff = {}
    o = 0
    for (D, noff, nw, soff) in windows:
        boff[(noff, nw)] = o
        o += 5 * nw

    per_core = []
    for c, co in enumerate(cores):
        src, dst, at, deg = co["src"], co["dst"], co["at"], co["deg"]
        np2s = np.array([_np2(max(int(d), 1)) for d in deg])
        rid = np.array([REG_DS.index(p) for p in np2s])
        order = np.argsort(rid, kind="stable")
        posmap = np.zeros(NLOC, np.int64)
        reg_fill = {D: 0 for D in REG_DS}
        for n in order:
            D = int(np2s[n])
            posmap[n] = base[D] + reg_fill[D]
            reg_fill[D] += 1

        eorder = np.argsort(dst, kind="stable")
        s_sorted = src[eorder]
        a_sorted = at[eorder]
        estart = np.zeros(NLOC + 1, np.int64)
        np.cumsum(np.bincount(dst, minlength=NLOC), out=estart[1:])

        node_at = np.full(W, -1, np.int64)
        node_at[posmap] = np.arange(NLOC)

        tok = np.full(TOT, W, np.int64)      # default: zero token
        oh_col = np.full(TOT, -1, np.int64)
        cpadv = np.zeros(W, np.float32)
        for (D, noff, nw, soff) in windows:
            nodes = node_at[noff:noff + nw]
            for j in range(nw):
                n = nodes[j]
                if n < 0:
                    cpadv[noff + j] = D
                    continue
                d = int(deg[n])
                cpadv[noff + j] = D - d
                if d == 0:
                    continue
                e0 = estart[n]
                for k in range(D):
                    e = e0 + (k if k < d else 0)
                    s = soff + k * nw + j
                    tok[s] = posmap[s_sorted[e]]
                    oh_col[s] = a_sorted[e]

        onehot = np.zeros((16, TOT), np.float16)
        valid = oh_col >= 0
        onehot[oh_col[valid], np.nonzero(valid)[0]] = 1.0

        dcl = np.maximum(deg.astype(np.float64), 1.0)
        ampv = np.zeros(W, np.float32)
        attv = np.zeros(W, np.float32)
        invv = np.ones(W, np.float32)
        mskv = np.zeros(W, np.float32)
        ampv[posmap] = (np.log(dcl + 1.0) / avg_log).astype(np.float32)
        attv[posmap] = (avg_log / np.log(dcl + 1.0)).astype(np.float32)
        invv[posmap] = (1.0 / dcl).astype(np.float32)
        mskv[posmap] = (deg > 0).astype(np.float32)

        bcw = np.zeros((128, 5 * W), np.float16)
        o = 0
        for (D, noff, nw, soff) in windows:
            blk = np.concatenate([
                ampv[noff:noff + nw], attv[noff:noff + nw],
                invv[noff:noff + nw], cpadv[noff:noff + nw],
                mskv[noff:noff + nw]])
            bcw[:, o:o + 5 * nw] = blk[None, :].astype(np.float16)
            o += 5 * nw

        xTf = np.zeros((8, W), np.float32)
        xTf[0:5, posmap] = x[c * NLOC:(c + 1) * NLOC].T
        xT = xTf.astype(np.float16)

        sg = np.zeros((W, GC), np.float16)
        gid = np.repeat(np.arange(GC), NPG)
        sg[posmap, gid] = 1.0

        per_core.append(dict(
            idx_tab=_wrap_idx_table(tok.astype(np.int16)),
            onehot=onehot, bcw=bcw, xT=xT, sg=sg))

    # ---- shared weights in device layouts ----
    def bf(a):
        return np.asarray(a, np.float32).astype(np.float16)

    emb1_w = np.asarray(inputs["emb1_w"], np.float32)
    pre_w = np.asarray(inputs["pre_w"], np.float32)
    post_w = np.asarray(inputs["post_w"], np.float32)
    edge_tab = np.asarray(inputs["edge_tab"], np.float32)
    enc_w = np.asarray(inputs["enc_w"], np.float32)
    enc_b = np.asarray(inputs["enc_b"], np.float32)
    pre_b = np.asarray(inputs["pre_b"], np.float32)

    embw = np.zeros((8, FP), np.float16)
    embw[0:5, 0:F] = bf(emb1_w)

    p12 = np.zeros((4, FP, 256), np.float16)
    qtab = np.zeros((4, 16, FP), np.float16)
    wblk = np.zeros((4, FP, 16 * FP), np.float16)
    for l in range(4):
        P1, P2, P3 = pre_w[l][0:F], pre_w[l][F:2 * F], pre_w[l][2 * F:3 * F]
        p12[l, 0:F, 0:F] = bf(P1)
        p12[l, 0:F, 128:128 + F] = bf(P2)
        q = (edge_tab @ enc_w[l] + enc_b[l]) @ P3 + pre_b[l]   # [10, F]
        qtab[l, 0:10, 0:F] = bf(q)
        Wh = post_w[l][0:F]
        blocks = [post_w[l][(1 + i) * F:(2 + i) * F] for i in range(12)]
        s_plain = blocks[0] + blocks[1] + blocks[2]
        s_amp = blocks[4] + blocks[5] + blocks[6]
        s_att = blocks[8] + blocks[9] + blocks[10]
        lw = np.asarray(inputs["lin_w"], np.float32)[l]
        for b, Wb in enumerate(blocks + [Wh, s_plain, s_amp, s_att]):
            wblk[l, 0:F, b * FP:b * FP + F] = bf(Wb @ lw)

    ident = np.zeros((FP, FP), np.float16)
    np.fill_diagonal(ident, 1.0)

    mw = np.zeros((FP, 176), np.float32)
    mw[0:F, 0:100] = np.asarray(inputs["mlp_w1"], np.float32)
    mw[0:100, 100:150] = np.asarray(inputs["mlp_w2"], np.float32)
    mw[0:50, 150:175] = np.asarray(inputs["mlp_w3"], np.float32)
    mw[0:25, 175:176] = np.asarray(inputs["mlp_w4"], np.float32)

    bcol = np.zeros((FP, 32), np.float32)
    bcol[0:F, 0] = np.asarray(inputs["emb1_b"], np.float32)
    for l in range(4):
        lwl = np.asarray(inputs["lin_w"], np.float32)[l]
        pb = np.asarray(inputs["post_b"], np.float32)[l]
        lb = np.asarray(inputs["lin_b"], np.float32)[l]
        bcol[0:F, 5 + l] = lwl.T @ pb + lb
        bcol[0:F, 9 + l] = np.asarray(inputs["bn_g"], np.float32)[l]
        bcol[0:F, 13 + l] = np.asarray(inputs["bn_b"], np.float32)[l]
    bcol[0:100, 17] = np.asarray(inputs["mlp_b1"], np.float32)
    bcol[0:50, 18] = np.asarray(inputs["mlp_b2"], np.float32)
    bcol[0:25, 19] = np.asarray(inputs["mlp_b3"], np.float32)
    bcol[0:1, 20] = np.asarray(inputs["mlp_b4"], np.float32)
    bcol[:, 21] = EPS

    shared = dict(embw=embw, p12=p12, qtab=qtab, wblk=wblk,
                  ident=ident, mw=mw, bcol=bcol)
    meta = dict(W=W, TOT=TOT, n_fake=n_fake, windows=windows, boff=boff)
    return meta, shared, per_core


def _build(meta):
    W, TOT = meta["W"], meta["TOT"]
    windows = meta["windows"]
    boff = meta["boff"]
    n_fake = meta["n_fake"]

    nc = bacc.Bacc("TRN2", target_bir_lowering=False, debug=False,
                   num_devices=NCORES)

    def inp(name, shape, dt):
        return nc.dram_tensor(name, shape, dt, kind="ExternalInput").ap()

    xT_in = inp("xT", [8, W], HF)
    idx_in = inp("idx", [128, TOT // 16], I16)
    oh_in = inp("oh", [16, TOT], HF)
    bcw_in = inp("bcw", [128, 5 * W], HF)
    sg_in = inp("sg", [W, GC], HF)
    embw_in = inp("embw", [8, FP], HF)
    p12_in = inp("p12", [4, FP, 256], HF)
    qtab_in = inp("qtab", [4, 16, FP], HF)
    wblk_in = inp("wblk", [4, FP, 16 * FP], HF)
    ident_in = inp("ident", [FP, FP], HF)
    mw_in = inp("mw", [FP, 176], F32)
    bcol_in = inp("bcol", [FP, 32], F32)
    out_ext = nc.dram_tensor("out", [1, GC], F32, kind="ExternalOutput").ap()

    AF = mybir.ActivationFunctionType
    AL = mybir.AluOpType

    with tile.TileContext(nc) as tc:
        with (
            tc.tile_pool(name="res", bufs=1) as res,
            tc.tile_pool(name="wl", bufs=1) as wl,
            tc.tile_pool(name="stream", bufs=2) as stream,
            tc.tile_pool(name="work", bufs=1) as work,
            tc.tile_pool(name="tree", bufs=1) as treep,
            tc.tile_pool(name="stat", bufs=1) as statp,
            tc.tile_pool(name="dram", bufs=1, space="DRAM") as dram,
        ):
            hT = res.tile([FP, W], HF, tag="hT")
            hp2 = res.tile([FP, W + FP], HF, tag="hp2")
            preBN = res.tile([FP, W], HF, tag="preBN")
            idx_sb = res.tile([128, TOT // 16], I16, tag="idx")
            ident_sb = res.tile([FP, FP], HF, tag="ident")
            bcol_sb = res.tile([FP, 32], F32, tag="bcol")
            scr = res.tile([FP, 192], F32, tag="scr")

            nc.sync.dma_start(idx_sb[:], idx_in)
            nc.sync.dma_start(ident_sb[:], ident_in)
            nc.sync.dma_start(bcol_sb[:], bcol_in)
            nc.vector.memset(hp2[:, W:W + FP], 0.0)

            with tc.tile_pool(name="ps", bufs=2, space="PSUM") as ps:
                # ---- h0 = x @ emb1_w + emb1_b ----
                with tc.tile_pool(name="x0", bufs=2) as x0p:
                    embw_sb = wl.tile([8, FP], HF, tag="embw")
                    nc.sync.dma_start(embw_sb[:], embw_in)
                    for s in range(0, W, 512):
                        sw = min(512, W - s)
                        xc = x0p.tile([8, 512], HF, tag="xc")
                        nc.sync.dma_start(xc[:, 0:sw], xT_in[:, s:s + sw])
                        p0 = ps.tile([FP, 512], F32, tag="pA")
                        nc.tensor.matmul(p0[:, 0:sw], embw_sb[:],
                                         xc[:, 0:sw], start=True, stop=True)
                        nc.scalar.activation(hT[:, s:s + sw], p0[:, 0:sw],
                                             AF.Identity, bias=bcol_sb[:, 0:1])

                for l in range(4):
                    p12_sb = wl.tile([FP, 256], HF, tag="p12")
                    q_sb = wl.tile([16, FP], HF, tag="q")
                    wblk_sb = wl.tile([FP, 16 * FP], HF, tag="wblk")
                    nc.sync.dma_start(p12_sb[:], p12_in[l])
                    nc.sync.dma_start(q_sb[:], qtab_in[l])
                    nc.sync.dma_start(wblk_sb[:], wblk_in[l])

                    # ---- Hp2 tokens ----
                    for s in range(0, W, 512):
                        sw = min(512, W - s)
                        p0 = ps.tile([FP, 512], F32, tag="pA")
                        for k in range(0, sw, 128):
                            nc.tensor.matmul(
                                p0[:, k:k + 128], hT[:, s + k:s + k + 128],
                                p12_sb[:, 128:256], start=True, stop=True)
                        nc.scalar.activation(hp2[:, s:s + sw], p0[:, 0:sw], AF.Copy)

                    nslc = 0
                    for (D, noff, nw, soff) in windows:
                        slots = D * nw
                        gat = work.tile([128, 1, SLOT_CAP], HF, tag="gat")
                        import os as _os
                        if _os.environ.get("K_NO_GATHER"):
                            nc.gpsimd.memset(gat[:, :, 0:slots], 0.125)
                        else:
                            nc.gpsimd.dma_gather(
                                gat[:, :, 0:slots], hp2[:],
                                idx_sb[:, soff // 16:(soff + slots) // 16],
                                slots, slots, elem_size=FP, transpose=True,
                                single_packet=False,
                                sbuf_tokens_per_rank=128, sbuf_free_dim_per_rank=256)
                        oh_sb = stream.tile([16, SLOT_CAP], HF, tag="oh")
                        nc.sync.dma_start(oh_sb[:, 0:slots],
                                          oh_in[:, soff:soff + slots])
                        bo = boff[(noff, nw)]
                        bc = stream.tile([128, 5 * 512], HF, tag="bc")
                        nc.sync.dma_start(bc[:, 0:5 * nw], bcw_in[:, bo:bo + 5 * nw])
                        amp_c = bc[:, 0 * nw:1 * nw]
                        att_c = bc[:, 1 * nw:2 * nw]
                        inv_c = bc[:, 2 * nw:3 * nw]
                        cpad_c = bc[:, 3 * nw:4 * nw]
                        msk_c = bc[:, 4 * nw:5 * nw]

                        # m = gathered + Q @ onehot ; msq = m^2
                        m_t = work.tile([128, SLOT_CAP], HF, tag="m")
                        msq_t = work.tile([128, SLOT_CAP], HF, tag="msq")
                        for s in range(0, slots, 512):
                            sw = min(512, slots - s)
                            p1 = ps.tile([FP, 512], F32, tag="pB")
                            nc.tensor.matmul(p1[:, 0:sw], ident_sb[:],
                                             gat[:, 0, s:s + sw],
                                             start=True, stop=False)
                            nc.tensor.matmul(p1[:, 0:sw], q_sb[:],
                                             oh_sb[:, s:s + sw],
                                             start=False, stop=True)
                            nc.scalar.activation(m_t[:, s:s + sw], p1[:, 0:sw],
                                                 AF.Copy)
                            nc.scalar.activation(msq_t[:, s:s + sw], p1[:, 0:sw],
                                                 AF.Square)

                        # halving trees over slot-major blocks -> (tile, off)
                        def tree(eng, op, buf, taga, tagb):
                            if D == 1:
                                return buf, 0
                            width = slots
                            cur, cur_off = buf, 0
                            use_a = True
                            while width > nw:
                                half = width // 2
                                dst = treep.tile([128, SLOT_CAP // 2], HF,
                                                 tag=(taga if use_a else tagb))
                                eng.tensor_tensor(
                                    dst[:, 0:half],
                                    cur[:, cur_off:cur_off + half],
                                    cur[:, cur_off + half:cur_off + width], op)
                                cur, cur_off, width = dst, 0, half
                                use_a = not use_a
                            return cur, 0

                        mx_t, mx_o = tree(nc.vector, AL.max, m_t, "tva", "tvb")
                        mn_t, mn_o = tree(nc.vector, AL.min, m_t, "tva2", "tvb2")
                        sm_t, sm_o = tree(nc.gpsimd, AL.add, m_t, "tga", "tgb")
                        sq_t, sq_o = tree(nc.gpsimd, AL.add, msq_t, "tga2", "tgb2")

                        # pad-replication correction: sum -= cpad * slot0
                        if D > 1:
                            t0 = statp.tile([128, 512], HF, tag="c0")
                            nc.gpsimd.tensor_tensor(t0[:, 0:nw], m_t[:, 0:nw],
                                                    cpad_c, AL.mult)
                            smc = statp.tile([128, 512], HF, tag="smc")
                            nc.gpsimd.tensor_tensor(smc[:, 0:nw],
                                                    sm_t[:, sm_o:sm_o + nw],
                                                    t0[:, 0:nw], AL.subtract)
                            sm_t, sm_o = smc, 0
                            t1 = statp.tile([128, 512], HF, tag="c1")
                            nc.gpsimd.tensor_tensor(t1[:, 0:nw], msq_t[:, 0:nw],
                                                    cpad_c, AL.mult)
                            sqc = statp.tile([128, 512], HF, tag="sqc")
                            nc.gpsimd.tensor_tensor(sqc[:, 0:nw],
                                                    sq_t[:, sq_o:sq_o + nw],
                                                    t1[:, 0:nw], AL.subtract)
                            sq_t, sq_o = sqc, 0

                        # mean / std
                        mean_s = statp.tile([128, 512], HF, tag="mean")
                        nc.vector.tensor_tensor(mean_s[:, 0:nw],
                                                sm_t[:, sm_o:sm_o + nw],
                                                inv_c, AL.mult)
                        msqm = statp.tile([128, 512], HF, tag="msqm")
                        nc.vector.tensor_tensor(msqm[:, 0:nw],
                                                sq_t[:, sq_o:sq_o + nw],
                                                inv_c, AL.mult)
                        var_s = statp.tile([128, 512], HF, tag="var")
                        nc.vector.tensor_tensor(var_s[:, 0:nw], mean_s[:, 0:nw],
                                                mean_s[:, 0:nw], AL.mult)
                        nc.vector.tensor_tensor(var_s[:, 0:nw], msqm[:, 0:nw],
                                                var_s[:, 0:nw], AL.subtract)
                        nc.vector.tensor_scalar_max(var_s[:, 0:nw],
                                                    var_s[:, 0:nw], 0.0)
                        std_s = statp.tile([128, 512], HF, tag="std")
                        nc.scalar.activation(std_s[:, 0:nw], var_s[:, 0:nw],
                                             AF.Sqrt, bias=bcol_sb[:, 21:22])

                        # Hp1 for this window
                        hp1ps = ps.tile([FP, 512], F32, tag="pC")
                        nc.tensor.matmul(hp1ps[:, 0:nw], p12_sb[:, 0:128],
                                         hT[:, noff:noff + nw],
                                         start=True, stop=True)
                        hp1m = statp.tile([128, 512], HF, tag="hp1m")
                        nc.scalar.activation(hp1m[:, 0:nw], hp1ps[:, 0:nw],
                                             AF.Copy)
                        nc.vector.tensor_tensor(hp1m[:, 0:nw], hp1m[:, 0:nw],
                                                msk_c, AL.mult)
                        hp1a = statp.tile([128, 512], HF, tag="hp1a")
                        nc.vector.tensor_tensor(hp1a[:, 0:nw], hp1m[:, 0:nw],
                                                amp_c, AL.mult)
                        hp1t = statp.tile([128, 512], HF, tag="hp1t")
                        nc.vector.tensor_tensor(hp1t[:, 0:nw], hp1m[:, 0:nw],
                                                att_c, AL.mult)

                        # amp/att-scaled stat blocks
                        raw = [(mean_s, 0), (mn_t, mn_o), (mx_t, mx_o), (std_s, 0)]
                        scaled = []
                        engs = [nc.vector, nc.gpsimd]
                        for i, bc_ap in enumerate([amp_c, att_c]):
                            for j, (bt, bo2) in enumerate(raw):
                                st = statp.tile([128, 512], HF, tag=f"sc{i}{j}")
                                engs[(i * 4 + j) % 2].tensor_tensor(
                                    st[:, 0:nw], bt[:, bo2:bo2 + nw], bc_ap,
                                    AL.mult)
                                scaled.append((st, 0))

                        blocks = raw + scaled + [
                            (hT, noff), (hp1m, 0), (hp1a, 0), (hp1t, 0)]

                        # post -> postT -> lin -> preBN (+ BN accums)
                        pp = ps.tile([FP, 512], F32, tag="pD")
                        for b, (bt, bo2) in enumerate(blocks):
                            nc.tensor.matmul(pp[:, 0:nw],
                                             wblk_sb[:, b * FP:(b + 1) * FP],
                                             bt[:, bo2:bo2 + nw],
                                             start=(b == 0), stop=(b == 15))
                        nc.scalar.activation(
                            preBN[:, noff:noff + nw], pp[:, 0:nw], AF.Identity,
                            bias=bcol_sb[:, 5 + l:6 + l],
                            accum_out=scr[:, nslc:nslc + 1])
                        sqt = statp.tile([128, 512], HF, tag="sqt")
                        nc.scalar.activation(
                            sqt[:, 0:nw], preBN[:, noff:noff + nw],
                            AF.Square, accum_out=scr[:, 64 + nslc:65 + nslc])
                        nslc += 1

                    # ---- BN (global over cores) ----
                    assert nslc <= 64
                    nc.vector.tensor_reduce(scr[:, 128:129], scr[:, 0:nslc],
                                            mybir.AxisListType.X, AL.add)
                    nc.vector.tensor_reduce(scr[:, 129:130],
                                            scr[:, 64:64 + nslc],
                                            mybir.AxisListType.X, AL.add)
                    ufake = preBN[:, W - 1:W]
                    nc.vector.tensor_scalar(out=scr[:, 130:131], in0=ufake,
                                            scalar1=float(n_fake), scalar2=None,
                                            op0=AL.mult)
                    nc.scalar.activation(scr[:, 131:132], ufake, AF.Square)
                    nc.vector.tensor_scalar(out=scr[:, 131:132],
                                            in0=scr[:, 131:132],
                                            scalar1=float(n_fake), scalar2=None,
                                            op0=AL.mult)
                    nc.vector.tensor_tensor(scr[:, 132:133], scr[:, 128:129],
                                            scr[:, 130:131], AL.subtract)
                    nc.vector.tensor_tensor(scr[:, 133:134], scr[:, 129:130],
                                            scr[:, 131:132], AL.subtract)

                    cc_in = dram.tile([FP, 2], F32, tag=f"ccin{l}")
                    cc_out = dram.tile([FP, 2], F32, tag=f"ccout{l}")
                    nc.gpsimd.dma_start(cc_in[:], scr[:, 132:134])
                    nc.gpsimd.collective_compute(
                        "AllReduce", AL.add,
                        replica_groups=[list(range(NCORES))],
                        ins=[cc_in.opt()], outs=[cc_out.opt()])
                    nc.sync.dma_start(scr[:, 134:136], cc_out[:])

                    nc.vector.tensor_scalar_mul(scr[:, 136:137],
                                                scr[:, 134:135], 1.0 / NTOT)
                    nc.vector.tensor_scalar_mul(scr[:, 137:138],
                                                scr[:, 135:136], 1.0 / NTOT)
                    nc.vector.tensor_tensor(scr[:, 138:139], scr[:, 136:137],
                                            scr[:, 136:137], AL.mult)
                    nc.vector.tensor_tensor(scr[:, 139:140], scr[:, 137:138],
                                            scr[:, 138:139], AL.subtract)
                    nc.scalar.activation(scr[:, 140:141], scr[:, 139:140],
                                         AF.Sqrt, bias=bcol_sb[:, 21:22])
                    nc.vector.reciprocal(scr[:, 141:142], scr[:, 140:141])
                    nc.vector.tensor_tensor(scr[:, 142:143], scr[:, 141:142],
                                            bcol_sb[:, 9 + l:10 + l], AL.mult)
                    nc.vector.tensor_tensor(scr[:, 143:144], scr[:, 136:137],
                                            scr[:, 142:143], AL.mult)
                    nc.vector.tensor_tensor(scr[:, 144:145],
                                            bcol_sb[:, 13 + l:14 + l],
                                            scr[:, 143:144], AL.subtract)

                    for s in range(0, W, 512):
                        sw = min(512, W - s)
                        nc.scalar.activation(hT[:, s:s + sw], preBN[:, s:s + sw],
                                             AF.Relu, bias=scr[:, 144:145],
                                             scale=scr[:, 142:143])

            # ---- pooling + MLP ----
            with tc.tile_pool(name="ps2", bufs=2, space="PSUM") as ps2:
                pooled_ps = ps2.tile([FP, GC], F32, tag="pool")
                nchunks = W // 128
                for t in range(nchunks):
                    tp = ps2.tile([FP, FP], HF, tag="ptr")
                    nc.tensor.transpose(tp[:], hT[:, t * 128:(t + 1) * 128],
                                        ident_sb[:])
                    hnm = work.tile([128, 128], HF, tag="hnm")
                    nc.scalar.activation(hnm[:], tp[:], AF.Copy)
                    sg_sb = stream.tile([128, GC], HF, tag="sg")
                    nc.sync.dma_start(sg_sb[:], sg_in[t * 128:(t + 1) * 128])
                    nc.tensor.matmul(pooled_ps[:], hnm[:], sg_sb[:],
                                     start=(t == 0), stop=(t == nchunks - 1))
                pooled = res.tile([FP, GC], F32, tag="pooled")
                nc.scalar.activation(pooled[:], pooled_ps[:], AF.Copy)

                mw_sb = wl.tile([FP, 176], F32, tag="mw")
                nc.sync.dma_start(mw_sb[:], mw_in)
                zp = ps2.tile([FP, GC], F32, tag="zp")
                z1 = res.tile([FP, GC], F32, tag="z1")
                nc.tensor.matmul(zp[0:100, :], mw_sb[0:128, 0:100], pooled[:],
                                 start=True, stop=True)
                nc.scalar.activation(z1[0:100, :], zp[0:100, :], AF.Relu,
                                     bias=bcol_sb[0:100, 17:18])
                zp2 = ps2.tile([FP, GC], F32, tag="zp")
                z2 = res.tile([FP, GC], F32, tag="z2")
                nc.tensor.matmul(zp2[0:50, :], mw_sb[0:100, 100:150],
                                 z1[0:100, :], start=True, stop=True)
                nc.scalar.activation(z2[0:50, :], zp2[0:50, :], AF.Relu,
                                     bias=bcol_sb[0:50, 18:19])
                zp3 = ps2.tile([FP, GC], F32, tag="zp")
                z3 = res.tile([FP, GC], F32, tag="z3")
                nc.tensor.matmul(zp3[0:25, :], mw_sb[0:50, 150:175],
                                 z2[0:50, :], start=True, stop=True)
                nc.scalar.activation(z3[0:25, :], zp3[0:25, :], AF.Relu,
                                     bias=bcol_sb[0:25, 19:20])
                zp4 = ps2.tile([FP, GC], F32, tag="zp")
                z4 = res.tile([1, GC], F32, tag="z4")
                nc.tensor.matmul(zp4[0:1, :], mw_sb[0:25, 175:176],
                                 z3[0:25, :], start=True, stop=True)
                nc.scalar.activation(z4[:], zp4[0:1, :], AF.Identity,
                                     bias=bcol_sb[0:1, 20:21])
                nc.sync.dma_start(out_ext, z4[:])

    nc.compile()
    return nc


def _execute(nc, in_maps, n_timing=3):
    """Run the compiled program via PJRT (axon); returns (results, best_ns)."""
    import jax
    from jax.sharding import Mesh, PartitionSpec
    from jax.experimental.shard_map import shard_map

    bass2jax.install_neuronx_cc_hook()
    partition_name = (nc.partition_id_tensor.name
                      if nc.partition_id_tensor else None)

    in_names, out_names, out_avals, zero_outs = [], [], [], []
    for alloc in nc.m.functions[0].allocations:
        if not isinstance(alloc, mybir.MemoryLocationSet):
            continue
        name = alloc.memorylocations[0].name
        if alloc.kind == "ExternalInput":
            if name != partition_name:
                in_names.append(name)
        elif alloc.kind == "ExternalOutput":
            out_names.append(name)
            shape = tuple(alloc.tensor_shape)
            dtype = mybir.dt.np(alloc.dtype)
            out_avals.append(jax.core.ShapedArray(shape, dtype))
            zero_outs.append(np.zeros(shape, dtype))
    n_params = len(in_names)
    n_outs = len(out_avals)
    all_names = in_names + out_names + ([partition_name] if partition_name else [])

    def _body(*args):
        operands = list(args)
        if partition_name is not None:
            operands.append(bass2jax.partition_id_tensor())
        outs = bass2jax._bass_exec_p.bind(
            *operands,
            out_avals=tuple(out_avals),
            in_names=tuple(all_names),
            out_names=tuple(out_names),
            lowering_input_output_aliases=(),
            sim_require_finite=False,
            sim_require_nnan=False,
            nc=nc,
        )
        return tuple(outs)

    devices = jax.devices()[:NCORES]
    mesh = Mesh(np.asarray(devices), ("core",))
    in_specs = (PartitionSpec("core"),) * (n_params + n_outs)
    out_specs = (PartitionSpec("core"),) * n_outs
    donate = tuple(range(n_params, n_params + n_outs))
    sharded = jax.jit(
        shard_map(_body, mesh=mesh, in_specs=in_specs, out_specs=out_specs,
                  check_rep=False),
        donate_argnums=donate, keep_unused=True)

    concat_in = [
        np.concatenate([np.asarray(in_maps[c][nm]) for c in range(NCORES)],
                       axis=0)
        for nm in in_names
    ]
    concat_in = [jax.device_put(a) for a in concat_in]

    def one_call():
        zs = [np.zeros((NCORES * z.shape[0], *z.shape[1:]), z.dtype)
              for z in zero_outs]
        outs = sharded(*concat_in, *zs)
        jax.block_until_ready(outs)
        return outs

    out_arrs = one_call()
    best = None
    for _ in range(n_timing):
        t0 = time.perf_counter()
        one_call()
        dt = time.perf_counter() - t0
        best = dt if best is None else min(best, dt)
    results = [
        {nm: np.asarray(out_arrs[i]).reshape(NCORES, *out_avals[i].shape)[c]
         for i, nm in enumerate(out_names)}
        for c in range(NCORES)
    ]
    return results, int((best or 0) * 1e9)


def kernel(**inputs):
    global LAST_HW_EXEC_NS
    meta, shared, per_core = _prep(inputs)
    nc = _build(meta)
    in_maps = []
    for c in range(NCORES):
        pc = per_core[c]
        in_maps.append(dict(
            xT=pc["xT"], idx=pc["idx_tab"], oh=pc["onehot"], bcw=pc["bcw"],
            sg=pc["sg"], embw=shared["embw"], p12=shared["p12"],
            qtab=shared["qtab"], wblk=shared["wblk"],
            ident=shared["ident"], mw=shared["mw"], bcol=shared["bcol"]))
    results, ns = _execute(nc, in_maps)
    LAST_HW_EXEC_NS = ns
    out = np.concatenate([results[c]["out"][0] for c in range(NCORES)])
    return out.reshape(G, 1).astype(np.float32)



# revision 3
# speedup vs baseline: 1.2945x; 1.2945x over previous
"""Trainium2 Bass kernel for the 4-layer PNA GNN (nn_Net_70025146794268).

Self-contained: accepts FULL inputs, shards graph-parallel over 8 NeuronCores,
runs a single SPMD Bass/Tile program, gathers the [G,1] output on host.

Per-core design (128 graphs, 16128 nodes, 57600 edges):
  * feature-major layouts throughout: tensors are [128 feat-partitions, nodes]
  * message m = Hp2[src] + Q[edge_attr] (pre_b folded into Q); the dst-side
    term Hp1[dst] is folded into the post matmul as extra K-blocks because
    mean/min/max shift by it and std is shift-invariant
  * per-node degree-padded slot tables (D in {1,2,4,8,16} regions) gathered via
    SBUF-source transpose dma_gather from the Hp2 token buffer; all four PNA
    stats computed by halving trees over the slot-major padded layout
  * BatchNorm is global over all 129024 nodes: per-core sums all-reduced via
    a tiny gpsimd collective each layer
  * pooling via per-graph 0/1 matmuls, then the readout MLP on-device in f32
"""

import time

import numpy as np
import ml_dtypes

import concourse.bacc as bacc
import concourse.tile as tile
import concourse.mybir as mybir
from concourse import bass2jax

HF = mybir.dt.float16
F32 = mybir.dt.float32
I16 = mybir.dt.int16

G, NPG, EPG, F = 1024, 126, 450, 126
NCORES = 8
GC = G // NCORES            # 128 graphs per core
NLOC = GC * NPG             # 16128 nodes per core
ELOC = GC * EPG             # 57600 edges per core
FP = 128                    # padded feature dim
EPS = 1e-5
NTOT = float(G * NPG)       # BN normalizer (all nodes, all cores)
SLOT_CAP = 2048             # max padded slots per window

LAST_HW_EXEC_NS = None


def _np2(d):
    p = 1
    while p < d:
        p *= 2
    return p


def _wrap_idx_table(flat):
    """[n] int16 -> [128, n//16] wrapped (i%16, i//16), replicated x8."""
    n = len(flat)
    assert n % 16 == 0
    tab = np.zeros((128, n // 16), np.int16)
    a = np.asarray(flat, np.int16).reshape(n // 16, 16).T  # [16, n//16]
    for g in range(8):
        tab[16 * g:16 * (g + 1)] = a
    return tab


def _prep(inputs):
    """Host-side preprocessing: common layout plan + per-core tables."""
    x = np.asarray(inputs["x"], np.float32)
    ei = np.asarray(inputs["edge_index"], np.int64)
    ea = np.asarray(inputs["edge_attr"], np.int64)
    dh = np.asarray(inputs["deg_hist"], np.float64)

    bins = np.arange(dh.shape[0], dtype=np.float64)
    avg_log = float((dh * np.log(bins + 1.0)).sum() / dh.sum())

    cores = []
    for c in range(NCORES):
        n0, e0 = c * NLOC, c * ELOC
        cores.append(dict(
            src=ei[0, e0:e0 + ELOC] - n0,
            dst=ei[1, e0:e0 + ELOC] - n0,
            at=ea[e0:e0 + ELOC],
        ))
        cores[-1]["deg"] = np.bincount(cores[-1]["dst"], minlength=NLOC)

    dmax = max(int(co["deg"].max()) for co in cores)
    if dmax > 32:
        raise RuntimeError(f"max degree {dmax} > 32 unsupported")
    REG_DS = [d for d in [32, 16, 8, 4, 2, 1] if d <= _np2(dmax)]

    # common padded region sizes (max over cores, 128-aligned)
    reg_counts = {D: 0 for D in REG_DS}
    for co in cores:
        np2s = np.array([_np2(max(int(d), 1)) for d in co["deg"]])
        for D in REG_DS:
            reg_counts[D] = max(reg_counts[D], int((np2s == D).sum()))
    npad = {}
    for D in REG_DS:
        n = -(-reg_counts[D] // 128) * 128
        if D == 1:
            n += 128          # guaranteed fake block (BN correction column)
        npad[D] = n

    windows = []              # (D, node_off, nw, slot_off)
    node_off, slot_off = 0, 0
    base = {}
    for D in REG_DS:
        base[D] = node_off
        win = max(128, min(512, SLOT_CAP // D))
        off = 0
        while off < npad[D]:
            nw = min(win, npad[D] - off)
            windows.append((D, node_off + off, nw, slot_off))
            slot_off += D * nw
            off += nw
        node_off += npad[D]
    W, TOT = node_off, slot_off
    n_fake = W - NLOC
    assert W % 128 == 0 and TOT % 128 == 0

    boff = {}
    o = 0
    for (D, noff, nw, soff) in windows:
        boff[(noff, nw)] = o
        o += 5 * nw

    per_core = []
    for c, co in enumerate(cores):
        src, dst, at, deg = co["src"], co["dst"], co["at"], co["deg"]
        np2s = np.array([_np2(max(int(d), 1)) for d in deg])
        rid = np.array([REG_DS.index(p) for p in np2s])
        order = np.argsort(rid, kind="stable")
        posmap = np.zeros(NLOC, np.int64)
        reg_fill = {D: 0 for D in REG_DS}
        for n in order:
            D = int(np2s[n])
            posmap[n] = base[D] + reg_fill[D]
            reg_fill[D] += 1

        eorder = np.argsort(dst, kind="stable")
        s_sorted = src[eorder]
        a_sorted = at[eorder]
        estart = np.zeros(NLOC + 1, np.int64)
        np.cumsum(np.bincount(dst, minlength=NLOC), out=estart[1:])

        node_at = np.full(W, -1, np.int64)
        node_at[posmap] = np.arange(NLOC)

        tok = np.full(TOT, W, np.int64)      # default: zero token
        oh_col = np.full(TOT, -1, np.int64)
        cpadv = np.zeros(W, np.float32)
        for (D, noff, nw, soff) in windows:
            nodes = node_at[noff:noff + nw]
            for j in range(nw):
                n = nodes[j]
                if n < 0:
                    cpadv[noff + j] = D
                    continue
                d = int(deg[n])
                cpadv[noff + j] = D - d
                if d == 0:
                    continue
                e0 = estart[n]
                for k in range(D):
                    e = e0 + (k if k < d else 0)
                    s = soff + k * nw + j
                    tok[s] = posmap[s_sorted[e]]
                    oh_col[s] = a_sorted[e]

        onehot = np.zeros((16, TOT), np.float16)
        valid = oh_col >= 0
        onehot[oh_col[valid], np.nonzero(valid)[0]] = 1.0

        dcl = np.maximum(deg.astype(np.float64), 1.0)
        ampv = np.zeros(W, np.float32)
        attv = np.zeros(W, np.float32)
        invv = np.ones(W, np.float32)
        mskv = np.zeros(W, np.float32)
        ampv[posmap] = (np.log(dcl + 1.0) / avg_log).astype(np.float32)
        attv[posmap] = (avg_log / np.log(dcl + 1.0)).astype(np.float32)
        invv[posmap] = (1.0 / dcl).astype(np.float32)
        mskv[posmap] = (deg > 0).astype(np.float32)

        bcw = np.zeros((128, 5 * W), np.float16)
        o = 0
        for (D, noff, nw, soff) in windows:
            blk = np.concatenate([
                ampv[noff:noff + nw], attv[noff:noff + nw],
                invv[noff:noff + nw], cpadv[noff:noff + nw],
                mskv[noff:noff + nw]])
            bcw[:, o:o + 5 * nw] = blk[None, :].astype(np.float16)
            o += 5 * nw

        xTf = np.zeros((8, W), np.float32)
        xTf[0:5, posmap] = x[c * NLOC:(c + 1) * NLOC].T
        xT = xTf.astype(np.float16)

        sg = np.zeros((W, GC), np.float16)
        gid = np.repeat(np.arange(GC), NPG)
        sg[posmap, gid] = 1.0

        per_core.append(dict(
            idx_tab=_wrap_idx_table(tok.astype(np.int16)),
            onehot=onehot, bcw=bcw, xT=xT, sg=sg))

    # ---- shared weights in device layouts ----
    def bf(a):
        return np.asarray(a, np.float32).astype(np.float16)

    emb1_w = np.asarray(inputs["emb1_w"], np.float32)
    pre_w = np.asarray(inputs["pre_w"], np.float32)
    post_w = np.asarray(inputs["post_w"], np.float32)
    edge_tab = np.asarray(inputs["edge_tab"], np.float32)
    enc_w = np.asarray(inputs["enc_w"], np.float32)
    enc_b = np.asarray(inputs["enc_b"], np.float32)
    pre_b = np.asarray(inputs["pre_b"], np.float32)

    embw = np.zeros((8, FP), np.float16)
    embw[0:5, 0:F] = bf(emb1_w)

    p12 = np.zeros((4, FP, 256), np.float16)
    qtab = np.zeros((4, 16, FP), np.float16)
    wblk = np.zeros((4, FP, 16 * FP), np.float16)
    for l in range(4):
        P1, P2, P3 = pre_w[l][0:F], pre_w[l][F:2 * F], pre_w[l][2 * F:3 * F]
        p12[l, 0:F, 0:F] = bf(P1)
        p12[l, 0:F, 128:128 + F] = bf(P2)
        q = (edge_tab @ enc_w[l] + enc_b[l]) @ P3 + pre_b[l]   # [10, F]
        qtab[l, 0:10, 0:F] = bf(q)
        Wh = post_w[l][0:F]
        blocks = [post_w[l][(1 + i) * F:(2 + i) * F] for i in range(12)]
        s_plain = blocks[0] + blocks[1] + blocks[2]
        s_amp = blocks[4] + blocks[5] + blocks[6]
        s_att = blocks[8] + blocks[9] + blocks[10]
        lw = np.asarray(inputs["lin_w"], np.float32)[l]
        for b, Wb in enumerate(blocks + [Wh, s_plain, s_amp, s_att]):
            wblk[l, 0:F, b * FP:b * FP + F] = bf(Wb @ lw)

    ident = np.zeros((FP, FP), np.float16)
    np.fill_diagonal(ident, 1.0)

    mw = np.zeros((FP, 176), np.float32)
    mw[0:F, 0:100] = np.asarray(inputs["mlp_w1"], np.float32)
    mw[0:100, 100:150] = np.asarray(inputs["mlp_w2"], np.float32)
    mw[0:50, 150:175] = np.asarray(inputs["mlp_w3"], np.float32)
    mw[0:25, 175:176] = np.asarray(inputs["mlp_w4"], np.float32)

    bcol = np.zeros((FP, 32), np.float32)
    bcol[0:F, 0] = np.asarray(inputs["emb1_b"], np.float32)
    for l in range(4):
        lwl = np.asarray(inputs["lin_w"], np.float32)[l]
        pb = np.asarray(inputs["post_b"], np.float32)[l]
        lb = np.asarray(inputs["lin_b"], np.float32)[l]
        bcol[0:F, 5 + l] = lwl.T @ pb + lb
        bcol[0:F, 9 + l] = np.asarray(inputs["bn_g"], np.float32)[l]
        bcol[0:F, 13 + l] = np.asarray(inputs["bn_b"], np.float32)[l]
    bcol[0:100, 17] = np.asarray(inputs["mlp_b1"], np.float32)
    bcol[0:50, 18] = np.asarray(inputs["mlp_b2"], np.float32)
    bcol[0:25, 19] = np.asarray(inputs["mlp_b3"], np.float32)
    bcol[0:1, 20] = np.asarray(inputs["mlp_b4"], np.float32)
    bcol[:, 21] = EPS

    shared = dict(embw=embw, p12=p12, qtab=qtab, wblk=wblk,
                  ident=ident, mw=mw, bcol=bcol)
    meta = dict(W=W, TOT=TOT, n_fake=n_fake, windows=windows, boff=boff)
    return meta, shared, per_core


def _build(meta):
    W, TOT = meta["W"], meta["TOT"]
    windows = meta["windows"]
    boff = meta["boff"]
    n_fake = meta["n_fake"]

    nc = bacc.Bacc("TRN2", target_bir_lowering=False, debug=False,
                   num_devices=NCORES)

    def inp(name, shape, dt):
        return nc.dram_tensor(name, shape, dt, kind="ExternalInput").ap()

    xT_in = inp("xT", [8, W], HF)
    idx_in = inp("idx", [128, TOT // 16], I16)
    oh_in = inp("oh", [16, TOT], HF)
    bcw_in = inp("bcw", [128, 5 * W], HF)
    sg_in = inp("sg", [W, GC], HF)
    embw_in = inp("embw", [8, FP], HF)
    p12_in = inp("p12", [4, FP, 256], HF)
    qtab_in = inp("qtab", [4, 16, FP], HF)
    wblk_in = inp("wblk", [4, FP, 16 * FP], HF)
    ident_in = inp("ident", [FP, FP], HF)
    mw_in = inp("mw", [FP, 176], F32)
    bcol_in = inp("bcol", [FP, 32], F32)
    out_ext = nc.dram_tensor("out", [1, GC], F32, kind="ExternalOutput").ap()

    AF = mybir.ActivationFunctionType
    AL = mybir.AluOpType

    with tile.TileContext(nc) as tc:
        with (
            tc.tile_pool(name="res", bufs=1) as res,
            tc.tile_pool(name="wl", bufs=1) as wl,
            tc.tile_pool(name="stream", bufs=2) as stream,
            tc.tile_pool(name="work", bufs=1) as work,
            tc.tile_pool(name="tree", bufs=1) as treep,
            tc.tile_pool(name="stat", bufs=1) as statp,
            tc.tile_pool(name="dram", bufs=1, space="DRAM") as dram,
        ):
            hT = res.tile([FP, W], HF, tag="hT")
            hp2 = res.tile([FP, W + FP], HF, tag="hp2")
            preBN = res.tile([FP, W], HF, tag="preBN")
            idx_sb = res.tile([128, TOT // 16], I16, tag="idx")
            ident_sb = res.tile([FP, FP], HF, tag="ident")
            bcol_sb = res.tile([FP, 32], F32, tag="bcol")
            scr = res.tile([FP, 192], F32, tag="scr")

            nc.sync.dma_start(idx_sb[:], idx_in)
            nc.sync.dma_start(ident_sb[:], ident_in)
            nc.sync.dma_start(bcol_sb[:], bcol_in)
            nc.vector.memset(hp2[:, W:W + FP], 0.0)

            with tc.tile_pool(name="ps", bufs=2, space="PSUM") as ps:
                # ---- h0 = x @ emb1_w + emb1_b ----
                with tc.tile_pool(name="x0", bufs=2) as x0p:
                    embw_sb = wl.tile([8, FP], HF, tag="embw")
                    nc.sync.dma_start(embw_sb[:], embw_in)
                    for s in range(0, W, 512):
                        sw = min(512, W - s)
                        xc = x0p.tile([8, 512], HF, tag="xc")
                        nc.sync.dma_start(xc[:, 0:sw], xT_in[:, s:s + sw])
                        p0 = ps.tile([FP, 512], F32, tag="pA")
                        nc.tensor.matmul(p0[:, 0:sw], embw_sb[:],
                                         xc[:, 0:sw], start=True, stop=True)
                        nc.scalar.activation(hT[:, s:s + sw], p0[:, 0:sw],
                                             AF.Identity, bias=bcol_sb[:, 0:1])

                for l in range(4):
                    p12_sb = wl.tile([FP, 256], HF, tag="p12")
                    q_sb = wl.tile([16, FP], HF, tag="q")
                    wblk_sb = wl.tile([FP, 16 * FP], HF, tag="wblk")
                    nc.sync.dma_start(p12_sb[:], p12_in[l])
                    nc.sync.dma_start(q_sb[:], qtab_in[l])
                    nc.sync.dma_start(wblk_sb[:], wblk_in[l])

                    # ---- Hp2 tokens ----
                    for s in range(0, W, 512):
                        sw = min(512, W - s)
                        p0 = ps.tile([FP, 512], F32, tag="pA")
                        for k in range(0, sw, 128):
                            nc.tensor.matmul(
                                p0[:, k:k + 128], hT[:, s + k:s + k + 128],
                                p12_sb[:, 128:256], start=True, stop=True)
                        nc.scalar.activation(hp2[:, s:s + sw], p0[:, 0:sw], AF.Copy)

                    nslc = 0
                    for (D, noff, nw, soff) in windows:
                        slots = D * nw
                        gat = work.tile([128, 1, SLOT_CAP], HF, tag="gat")
                        import os as _os
                        if _os.environ.get("K_NO_GATHER"):
                            nc.gpsimd.memset(gat[:, :, 0:slots], 0.125)
                        else:
                            nc.gpsimd.dma_gather(
                                gat[:, :, 0:slots], hp2[:],
                                idx_sb[:, soff // 16:(soff + slots) // 16],
                                slots, slots, elem_size=FP, transpose=True,
                                single_packet=False,
                                sbuf_tokens_per_rank=128, sbuf_free_dim_per_rank=256)
                        oh_sb = stream.tile([16, SLOT_CAP], HF, tag="oh")
                        nc.sync.dma_start(oh_sb[:, 0:slots],
                                          oh_in[:, soff:soff + slots])
                        bo = boff[(noff, nw)]
                        bc = stream.tile([128, 5 * 512], HF, tag="bc")
                        nc.sync.dma_start(bc[:, 0:5 * nw], bcw_in[:, bo:bo + 5 * nw])
                        amp_c = bc[:, 0 * nw:1 * nw]
                        att_c = bc[:, 1 * nw:2 * nw]
                        inv_c = bc[:, 2 * nw:3 * nw]
                        cpad_c = bc[:, 3 * nw:4 * nw]
                        msk_c = bc[:, 4 * nw:5 * nw]

                        # m = gathered + Q @ onehot ; msq = m^2
                        m_t = work.tile([128, SLOT_CAP], HF, tag="m")
                        msq_t = work.tile([128, SLOT_CAP], HF, tag="msq")
                        for s in range(0, slots, 512):
                            sw = min(512, slots - s)
                            p1 = ps.tile([FP, 512], F32, tag="pB")
                            nc.tensor.matmul(p1[:, 0:sw], ident_sb[:],
                                             gat[:, 0, s:s + sw],
                                             start=True, stop=False)
                            nc.tensor.matmul(p1[:, 0:sw], q_sb[:],
                                             oh_sb[:, s:s + sw],
                                             start=False, stop=True)
                            nc.scalar.activation(m_t[:, s:s + sw], p1[:, 0:sw],
                                                 AF.Copy)
                            nc.scalar.activation(msq_t[:, s:s + sw], p1[:, 0:sw],
                                                 AF.Square)

                        # halving trees over slot-major blocks -> (tile, off)
                        def tree(eng, op, buf, taga, tagb):
                            if D == 1:
                                return buf, 0
                            width = slots
                            cur, cur_off = buf, 0
                            use_a = True
                            while width > nw:
                                half = width // 2
                                dst = treep.tile([128, SLOT_CAP // 2], HF,
                                                 tag=(taga if use_a else tagb))
                                eng.tensor_tensor(
                                    dst[:, 0:half],
                                    cur[:, cur_off:cur_off + half],
                                    cur[:, cur_off + half:cur_off + width], op)
                                cur, cur_off, width = dst, 0, half
                                use_a = not use_a
                            return cur, 0

                        mx_t, mx_o = tree(nc.vector, AL.max, m_t, "tva", "tvb")
                        mn_t, mn_o = tree(nc.vector, AL.min, m_t, "tva2", "tvb2")
                        sm_t, sm_o = tree(nc.gpsimd, AL.add, m_t, "tga", "tgb")
                        sq_t, sq_o = tree(nc.gpsimd, AL.add, msq_t, "tga2", "tgb2")

                        # pad-replication correction: sum -= cpad * slot0
                        if D > 1:
                            t0 = statp.tile([128, 512], HF, tag="c0")
                            nc.gpsimd.tensor_tensor(t0[:, 0:nw], m_t[:, 0:nw],
                                                    cpad_c, AL.mult)
                            smc = statp.tile([128, 512], HF, tag="smc")
                            nc.gpsimd.tensor_tensor(smc[:, 0:nw],
                                                    sm_t[:, sm_o:sm_o + nw],
                                                    t0[:, 0:nw], AL.subtract)
                            sm_t, sm_o = smc, 0
                            t1 = statp.tile([128, 512], HF, tag="c1")
                            nc.gpsimd.tensor_tensor(t1[:, 0:nw], msq_t[:, 0:nw],
                                                    cpad_c, AL.mult)
                            sqc = statp.tile([128, 512], HF, tag="sqc")
                            nc.gpsimd.tensor_tensor(sqc[:, 0:nw],
                                                    sq_t[:, sq_o:sq_o + nw],
                                                    t1[:, 0:nw], AL.subtract)
                            sq_t, sq_o = sqc, 0

                        # mean / std
                        mean_s = statp.tile([128, 512], HF, tag="mean")
                        nc.vector.tensor_tensor(mean_s[:, 0:nw],
                                                sm_t[:, sm_o:sm_o + nw],
                                                inv_c, AL.mult)
                        msqm = statp.tile([128, 512], HF, tag="msqm")
                        nc.vector.tensor_tensor(msqm[:, 0:nw],
                                                sq_t[:, sq_o:sq_o + nw],
                                                inv_c, AL.mult)
                        var_s = statp.tile([128, 512], HF, tag="var")
                        nc.vector.tensor_tensor(var_s[:, 0:nw], mean_s[:, 0:nw],
                                                mean_s[:, 0:nw], AL.mult)
                        nc.vector.tensor_tensor(var_s[:, 0:nw], msqm[:, 0:nw],
                                                var_s[:, 0:nw], AL.subtract)
                        nc.vector.tensor_scalar_max(var_s[:, 0:nw],
                                                    var_s[:, 0:nw], 0.0)
                        std_s = statp.tile([128, 512], HF, tag="std")
                        nc.scalar.activation(std_s[:, 0:nw], var_s[:, 0:nw],
                                             AF.Sqrt, bias=bcol_sb[:, 21:22])

                        # Hp1 for this window
                        hp1ps = ps.tile([FP, 512], F32, tag="pC")
                        nc.tensor.matmul(hp1ps[:, 0:nw], p12_sb[:, 0:128],
                                         hT[:, noff:noff + nw],
                                         start=True, stop=True)
                        hp1m = statp.tile([128, 512], HF, tag="hp1m")
                        nc.scalar.activation(hp1m[:, 0:nw], hp1ps[:, 0:nw],
                                             AF.Copy)
                        nc.vector.tensor_tensor(hp1m[:, 0:nw], hp1m[:, 0:nw],
                                                msk_c, AL.mult)
                        hp1a = statp.tile([128, 512], HF, tag="hp1a")
                        nc.vector.tensor_tensor(hp1a[:, 0:nw], hp1m[:, 0:nw],
                                                amp_c, AL.mult)
                        hp1t = statp.tile([128, 512], HF, tag="hp1t")
                        nc.vector.tensor_tensor(hp1t[:, 0:nw], hp1m[:, 0:nw],
                                                att_c, AL.mult)

                        # amp/att-scaled stat blocks
                        raw = [(mean_s, 0), (mn_t, mn_o), (mx_t, mx_o), (std_s, 0)]
                        scaled = []
                        engs = [nc.vector, nc.gpsimd]
                        for i, bc_ap in enumerate([amp_c, att_c]):
                            for j, (bt, bo2) in enumerate(raw):
                                st = statp.tile([128, 512], HF, tag=f"sc{i}{j}")
                                engs[(i * 4 + j) % 2].tensor_tensor(
                                    st[:, 0:nw], bt[:, bo2:bo2 + nw], bc_ap,
                                    AL.mult)
                                scaled.append((st, 0))

                        blocks = raw + scaled + [
                            (hT, noff), (hp1m, 0), (hp1a, 0), (hp1t, 0)]

                        # post -> postT -> lin -> preBN (+ BN accums)
                        pp = ps.tile([FP, 512], F32, tag="pD")
                        for b, (bt, bo2) in enumerate(blocks):
                            nc.tensor.matmul(pp[:, 0:nw],
                                             wblk_sb[:, b * FP:(b + 1) * FP],
                                             bt[:, bo2:bo2 + nw],
                                             start=(b == 0), stop=(b == 15))
                        nc.scalar.activation(
                            preBN[:, noff:noff + nw], pp[:, 0:nw], AF.Identity,
                            bias=bcol_sb[:, 5 + l:6 + l],
                            accum_out=scr[:, nslc:nslc + 1])
                        sqt = statp.tile([128, 512], HF, tag="sqt")
                        nc.scalar.activation(
                            sqt[:, 0:nw], preBN[:, noff:noff + nw],
                            AF.Square, accum_out=scr[:, 64 + nslc:65 + nslc])
                        nslc += 1

                    # ---- BN (global over cores) ----
                    assert nslc <= 64
                    nc.vector.tensor_reduce(scr[:, 128:129], scr[:, 0:nslc],
                                            mybir.AxisListType.X, AL.add)
                    nc.vector.tensor_reduce(scr[:, 129:130],
                                            scr[:, 64:64 + nslc],
                                            mybir.AxisListType.X, AL.add)
                    ufake = preBN[:, W - 1:W]
                    nc.vector.tensor_scalar(out=scr[:, 130:131], in0=ufake,
                                            scalar1=float(n_fake), scalar2=None,
                                            op0=AL.mult)
                    nc.scalar.activation(scr[:, 131:132], ufake, AF.Square)
                    nc.vector.tensor_scalar(out=scr[:, 131:132],
                                            in0=scr[:, 131:132],
                                            scalar1=float(n_fake), scalar2=None,
                                            op0=AL.mult)
                    nc.vector.tensor_tensor(scr[:, 132:133], scr[:, 128:129],
                                            scr[:, 130:131], AL.subtract)
                    nc.vector.tensor_tensor(scr[:, 133:134], scr[:, 129:130],
                                            scr[:, 131:132], AL.subtract)

                    cc_in = dram.tile([FP, 2], F32, tag=f"ccin{l}")
                    cc_out = dram.tile([FP, 2], F32, tag=f"ccout{l}")
                    nc.gpsimd.dma_start(cc_in[:], scr[:, 132:134])
                    nc.gpsimd.collective_compute(
                        "AllReduce", AL.add,
                        replica_groups=[list(range(NCORES))],
                        ins=[cc_in.opt()], outs=[cc_out.opt()])
                    nc.sync.dma_start(scr[:, 134:136], cc_out[:])

                    nc.vector.tensor_scalar_mul(scr[:, 136:137],
                                                scr[:, 134:135], 1.0 / NTOT)
                    nc.vector.tensor_scalar_mul(scr[:, 137:138],
                                                scr[:, 135:136], 1.0 / NTOT)
                    nc.vector.tensor_tensor(scr[:, 138:139], scr[:, 136:137],
                                            scr[:, 136:137], AL.mult)
                    nc.vector.tensor_tensor(scr[:, 139:140], scr[:, 137:138],
                                            scr[:, 138:139], AL.subtract)
                    nc.scalar.activation(scr[:, 140:141], scr[:, 139:140],
                                         AF.Sqrt, bias=bcol_sb[:, 21:22])
                    nc.vector.reciprocal(scr[:, 141:142], scr[:, 140:141])
                    nc.vector.tensor_tensor(scr[:, 142:143], scr[:, 141:142],
                                            bcol_sb[:, 9 + l:10 + l], AL.mult)
                    nc.vector.tensor_tensor(scr[:, 143:144], scr[:, 136:137],
                                            scr[:, 142:143], AL.mult)
                    nc.vector.tensor_tensor(scr[:, 144:145],
                                            bcol_sb[:, 13 + l:14 + l],
                                            scr[:, 143:144], AL.subtract)

                    for s in range(0, W, 512):
                        sw = min(512, W - s)
                        nc.scalar.activation(hT[:, s:s + sw], preBN[:, s:s + sw],
                                             AF.Relu, bias=scr[:, 144:145],
                                             scale=scr[:, 142:143])

            # ---- pooling + MLP ----
            with tc.tile_pool(name="ps2", bufs=2, space="PSUM") as ps2:
                pooled_ps = ps2.tile([FP, GC], F32, tag="pool")
                nchunks = W // 128
                for t in range(nchunks):
                    tp = ps2.tile([FP, FP], HF, tag="ptr")
                    nc.tensor.transpose(tp[:], hT[:, t * 128:(t + 1) * 128],
                                        ident_sb[:])
                    hnm = work.tile([128, 128], HF, tag="hnm")
                    nc.scalar.activation(hnm[:], tp[:], AF.Copy)
                    sg_sb = stream.tile([128, GC], HF, tag="sg")
                    nc.sync.dma_start(sg_sb[:], sg_in[t * 128:(t + 1) * 128])
                    nc.tensor.matmul(pooled_ps[:], hnm[:], sg_sb[:],
                                     start=(t == 0), stop=(t == nchunks - 1))
                pooled = res.tile([FP, GC], F32, tag="pooled")
                nc.scalar.activation(pooled[:], pooled_ps[:], AF.Copy)

                mw_sb = wl.tile([FP, 176], F32, tag="mw")
                nc.sync.dma_start(mw_sb[:], mw_in)
                zp = ps2.tile([FP, GC], F32, tag="zp")
                z1 = res.tile([FP, GC], F32, tag="z1")
                nc.tensor.matmul(zp[0:100, :], mw_sb[0:128, 0:100], pooled[:],
                                 start=True, stop=True)
                nc.scalar.activation(z1[0:100, :], zp[0:100, :], AF.Relu,
                                     bias=bcol_sb[0:100, 17:18])
                zp2 = ps2.tile([FP, GC], F32, tag="zp")
                z2 = res.tile([FP, GC], F32, tag="z2")
                nc.tensor.matmul(zp2[0:50, :], mw_sb[0:100, 100:150],
                                 z1[0:100, :], start=True, stop=True)
                nc.scalar.activation(z2[0:50, :], zp2[0:50, :], AF.Relu,
                                     bias=bcol_sb[0:50, 18:19])
                zp3 = ps2.tile([FP, GC], F32, tag="zp")
                z3 = res.tile([FP, GC], F32, tag="z3")
                nc.tensor.matmul(zp3[0:25, :], mw_sb[0:50, 150:175],
                                 z2[0:50, :], start=True, stop=True)
                nc.scalar.activation(z3[0:25, :], zp3[0:25, :], AF.Relu,
                                     bias=bcol_sb[0:25, 19:20])
                zp4 = ps2.tile([FP, GC], F32, tag="zp")
                z4 = res.tile([1, GC], F32, tag="z4")
                nc.tensor.matmul(zp4[0:1, :], mw_sb[0:25, 175:176],
                                 z3[0:25, :], start=True, stop=True)
                nc.scalar.activation(z4[:], zp4[0:1, :], AF.Identity,
                                     bias=bcol_sb[0:1, 20:21])
                nc.sync.dma_start(out_ext, z4[:])

    nc.compile()
    return nc


def _execute(nc, in_maps, n_timing=3):
    """Run the compiled program via PJRT (axon); returns (results, best_ns)."""
    import jax
    from jax.sharding import Mesh, PartitionSpec, NamedSharding
    from jax.experimental.shard_map import shard_map

    bass2jax.install_neuronx_cc_hook()
    partition_name = (nc.partition_id_tensor.name
                      if nc.partition_id_tensor else None)

    in_names, out_names, out_avals, zero_outs = [], [], [], []
    for alloc in nc.m.functions[0].allocations:
        if not isinstance(alloc, mybir.MemoryLocationSet):
            continue
        name = alloc.memorylocations[0].name
        if alloc.kind == "ExternalInput":
            if name != partition_name:
                in_names.append(name)
        elif alloc.kind == "ExternalOutput":
            out_names.append(name)
            shape = tuple(alloc.tensor_shape)
            dtype = mybir.dt.np(alloc.dtype)
            out_avals.append(jax.core.ShapedArray(shape, dtype))
            zero_outs.append(np.zeros(shape, dtype))
    n_params = len(in_names)
    n_outs = len(out_avals)
    all_names = in_names + out_names + ([partition_name] if partition_name else [])

    def _body(*args):
        operands = list(args)
        if partition_name is not None:
            operands.append(bass2jax.partition_id_tensor())
        outs = bass2jax._bass_exec_p.bind(
            *operands,
            out_avals=tuple(out_avals),
            in_names=tuple(all_names),
            out_names=tuple(out_names),
            lowering_input_output_aliases=(),
            sim_require_finite=False,
            sim_require_nnan=False,
            nc=nc,
        )
        return tuple(outs)

    devices = jax.devices()[:NCORES]
    mesh = Mesh(np.asarray(devices), ("core",))
    in_specs = (PartitionSpec("core"),) * (n_params + n_outs)
    out_specs = (PartitionSpec("core"),) * n_outs
    donate = tuple(range(n_params, n_params + n_outs))
    sharded = jax.jit(
        shard_map(_body, mesh=mesh, in_specs=in_specs, out_specs=out_specs,
                  check_rep=False),
        donate_argnums=donate, keep_unused=True)

    # Pre-shard inputs onto the mesh: without an explicit NamedSharding the
    # jit inserts a per-argument reshard executable on EVERY call (12 extra
    # device round trips per invocation, ~100 ms of dispatch overhead).
    shard = NamedSharding(mesh, PartitionSpec("core"))
    concat_in = [
        np.concatenate([np.asarray(in_maps[c][nm]) for c in range(NCORES)],
                       axis=0)
        for nm in in_names
    ]
    concat_in = [jax.device_put(a, shard) for a in concat_in]

    def one_call():
        zs = [jax.device_put(
                  np.zeros((NCORES * z.shape[0], *z.shape[1:]), z.dtype), shard)
              for z in zero_outs]
        outs = sharded(*concat_in, *zs)
        jax.block_until_ready(outs)
        return outs

    out_arrs = one_call()
    best = None
    for _ in range(n_timing):
        t0 = time.perf_counter()
        one_call()
        dt = time.perf_counter() - t0
        best = dt if best is None else min(best, dt)
    results = [
        {nm: np.asarray(out_arrs[i]).reshape(NCORES, *out_avals[i].shape)[c]
         for i, nm in enumerate(out_names)}
        for c in range(NCORES)
    ]
    return results, int((best or 0) * 1e9)


def kernel(**inputs):
    global LAST_HW_EXEC_NS
    meta, shared, per_core = _prep(inputs)
    nc = _build(meta)
    in_maps = []
    for c in range(NCORES):
        pc = per_core[c]
        in_maps.append(dict(
            xT=pc["xT"], idx=pc["idx_tab"], oh=pc["onehot"], bcw=pc["bcw"],
            sg=pc["sg"], embw=shared["embw"], p12=shared["p12"],
            qtab=shared["qtab"], wblk=shared["wblk"],
            ident=shared["ident"], mw=shared["mw"], bcol=shared["bcol"]))
    results, ns = _execute(nc, in_maps)
    LAST_HW_EXEC_NS = ns
    out = np.concatenate([results[c]["out"][0] for c in range(NCORES)])
    return out.reshape(G, 1).astype(np.float32)



# revision 9
# speedup vs baseline: 1.3658x; 1.0551x over previous
"""Trainium2 Bass kernel for the 4-layer PNA GNN (nn_Net_70025146794268).

Self-contained: accepts FULL inputs, shards graph-parallel over 8 NeuronCores,
runs a single SPMD Bass/Tile program, gathers the [G,1] output on host.

Per-core design (128 graphs, 16128 nodes, 57600 edges):
  * feature-major layouts throughout: tensors are [128 feat-partitions, nodes]
  * message m = Hp2[src] + Q[edge_attr] (pre_b folded into Q); the dst-side
    term Hp1[dst] is folded into the post matmul as extra K-blocks because
    mean/min/max shift by it and std is shift-invariant
  * per-node degree-padded slot tables (D in {1,2,4,8,16} regions) gathered via
    SBUF-source transpose dma_gather from the Hp2 token buffer; all four PNA
    stats computed by halving trees over the slot-major padded layout
  * the Q[edge_attr] term is precomputed host-side per layer (qoh) and
    streamed from HBM, so the message add is one vector op per window
  * GpSimd runs ONLY the gathers (single ucode library, no MPC reloads);
    all tensor_tensor work lives on the Vector engine
  * post_nn/lin matmuls are regrouped into three PSUM groups (plain/amp/att);
    the per-node degree scalers multiply the group outputs, which removes the
    10 per-window broadcast multiplies the naive layout needs
  * BatchNorm is global over all 129024 nodes: per-core sums all-reduced via
    a tiny gpsimd collective each layer
  * pooling via per-graph 0/1 matmuls, then the readout MLP on-device in f32
"""

import time

import numpy as np
import ml_dtypes

import concourse.bacc as bacc
import concourse.tile as tile
import concourse.mybir as mybir
from concourse import bass2jax

HF = mybir.dt.float16
F32 = mybir.dt.float32
I16 = mybir.dt.int16

G, NPG, EPG, F = 1024, 126, 450, 126
NCORES = 8
GC = G // NCORES            # 128 graphs per core
NLOC = GC * NPG             # 16128 nodes per core
ELOC = GC * EPG             # 57600 edges per core
FP = 128                    # padded feature dim
EPS = 1e-5
NTOT = float(G * NPG)       # BN normalizer (all nodes, all cores)
SLOT_CAP = 4096             # max padded slots per window
NW_CAP = 1024               # max nodes per window

LAST_HW_EXEC_NS = None


def _np2(d):
    p = 1
    while p < d:
        p *= 2
    return p


def _wrap_idx_table(flat):
    """[n] int16 -> [128, n//16] wrapped (i%16, i//16), replicated x8."""
    n = len(flat)
    assert n % 16 == 0
    tab = np.zeros((128, n // 16), np.int16)
    a = np.asarray(flat, np.int16).reshape(n // 16, 16).T  # [16, n//16]
    for g in range(8):
        tab[16 * g:16 * (g + 1)] = a
    return tab


def _prep(inputs):
    """Host-side preprocessing: common layout plan + per-core tables."""
    x = np.asarray(inputs["x"], np.float32)
    ei = np.asarray(inputs["edge_index"], np.int64)
    ea = np.asarray(inputs["edge_attr"], np.int64)
    dh = np.asarray(inputs["deg_hist"], np.float64)

    bins = np.arange(dh.shape[0], dtype=np.float64)
    avg_log = float((dh * np.log(bins + 1.0)).sum() / dh.sum())

    cores = []
    for c in range(NCORES):
        n0, e0 = c * NLOC, c * ELOC
        cores.append(dict(
            src=ei[0, e0:e0 + ELOC] - n0,
            dst=ei[1, e0:e0 + ELOC] - n0,
            at=ea[e0:e0 + ELOC],
        ))
        cores[-1]["deg"] = np.bincount(cores[-1]["dst"], minlength=NLOC)

    dmax = max(int(co["deg"].max()) for co in cores)
    if dmax > 32:
        raise RuntimeError(f"max degree {dmax} > 32 unsupported")
    REG_DS = [d for d in [32, 16, 8, 4, 2, 1] if d <= _np2(dmax)]

    # common padded region sizes (max over cores, 128-aligned)
    reg_counts = {D: 0 for D in REG_DS}
    for co in cores:
        np2s = np.array([_np2(max(int(d), 1)) for d in co["deg"]])
        for D in REG_DS:
            reg_counts[D] = max(reg_counts[D], int((np2s == D).sum()))
    npad = {}
    for D in REG_DS:
        n = -(-reg_counts[D] // 128) * 128
        if D == 1:
            n += 128          # guaranteed fake block (BN correction column)
        npad[D] = n

    windows = []              # (D, node_off, nw, slot_off)
    node_off, slot_off = 0, 0
    base = {}
    for D in REG_DS:
        base[D] = node_off
        win = max(128, min(NW_CAP, SLOT_CAP // D))
        off = 0
        while off < npad[D]:
            nw = min(win, npad[D] - off)
            windows.append((D, node_off + off, nw, slot_off))
            slot_off += D * nw
            off += nw
        node_off += npad[D]
    W, TOT = node_off, slot_off
    n_fake = W - NLOC
    assert W % 128 == 0 and TOT % 128 == 0

    boff = {}
    o = 0
    for (D, noff, nw, soff) in windows:
        boff[(noff, nw)] = o
        o += 5 * nw

    per_core = []
    for c, co in enumerate(cores):
        src, dst, at, deg = co["src"], co["dst"], co["at"], co["deg"]
        np2s = np.array([_np2(max(int(d), 1)) for d in deg])
        rid = np.array([REG_DS.index(p) for p in np2s])
        order = np.argsort(rid, kind="stable")
        posmap = np.zeros(NLOC, np.int64)
        reg_fill = {D: 0 for D in REG_DS}
        for n in order:
            D = int(np2s[n])
            posmap[n] = base[D] + reg_fill[D]
            reg_fill[D] += 1

        eorder = np.argsort(dst, kind="stable")
        s_sorted = src[eorder]
        a_sorted = at[eorder]
        estart = np.zeros(NLOC + 1, np.int64)
        np.cumsum(np.bincount(dst, minlength=NLOC), out=estart[1:])

        node_at = np.full(W, -1, np.int64)
        node_at[posmap] = np.arange(NLOC)

        tok = np.full(TOT, W, np.int64)      # default: zero token
        oh_col = np.full(TOT, -1, np.int64)
        cpadv = np.zeros(W, np.float32)
        for (D, noff, nw, soff) in windows:
            nodes = node_at[noff:noff + nw]
            for j in range(nw):
                n = nodes[j]
                if n < 0:
                    cpadv[noff + j] = D
                    continue
                d = int(deg[n])
                cpadv[noff + j] = D - d
                if d == 0:
                    continue
                e0 = estart[n]
                for k in range(D):
                    e = e0 + (k if k < d else 0)
                    s = soff + k * nw + j
                    tok[s] = posmap[s_sorted[e]]
                    oh_col[s] = a_sorted[e]

        dcl = np.maximum(deg.astype(np.float64), 1.0)
        ampv = np.zeros(W, np.float32)
        attv = np.zeros(W, np.float32)
        invv = np.ones(W, np.float32)
        mskv = np.zeros(W, np.float32)
        ampv[posmap] = (np.log(dcl + 1.0) / avg_log).astype(np.float32)
        attv[posmap] = (avg_log / np.log(dcl + 1.0)).astype(np.float32)
        invv[posmap] = (1.0 / dcl).astype(np.float32)
        mskv[posmap] = (deg > 0).astype(np.float32)

        bcw = np.zeros((128, 5 * W), np.float16)
        o = 0
        for (D, noff, nw, soff) in windows:
            blk = np.concatenate([
                ampv[noff:noff + nw], attv[noff:noff + nw],
                invv[noff:noff + nw], cpadv[noff:noff + nw],
                mskv[noff:noff + nw]])
            bcw[:, o:o + 5 * nw] = blk[None, :].astype(np.float16)
            o += 5 * nw

        xTf = np.zeros((8, W), np.float32)
        xTf[0:5, posmap] = x[c * NLOC:(c + 1) * NLOC].T
        xT = xTf.astype(np.float16)

        sg = np.zeros((W, GC), np.float16)
        gid = np.repeat(np.arange(GC), NPG)
        sg[posmap, gid] = 1.0

        per_core.append(dict(
            idx_tab=_wrap_idx_table(tok.astype(np.int16)),
            oh_col=oh_col, bcw=bcw, xT=xT, sg=sg))

    # ---- shared weights in device layouts ----
    def bf(a):
        return np.asarray(a, np.float32).astype(np.float16)

    emb1_w = np.asarray(inputs["emb1_w"], np.float32)
    pre_w = np.asarray(inputs["pre_w"], np.float32)
    post_w = np.asarray(inputs["post_w"], np.float32)
    edge_tab = np.asarray(inputs["edge_tab"], np.float32)
    enc_w = np.asarray(inputs["enc_w"], np.float32)
    enc_b = np.asarray(inputs["enc_b"], np.float32)
    pre_b = np.asarray(inputs["pre_b"], np.float32)

    embw = np.zeros((8, FP), np.float16)
    embw[0:5, 0:F] = bf(emb1_w)

    p12 = np.zeros((4, FP, 256), np.float16)
    qtabs = []
    # wblk block order (16 blocks of FP cols):
    #   plain group: [W0 W1 W2 W3] (mean,mn,mx,std), s_plain, Wh
    #   amp   group: [W4..W7], s_amp
    #   att   group: [W8..W11], s_att
    wblk = np.zeros((4, FP, 16 * FP), np.float16)
    for l in range(4):
        P1, P2, P3 = pre_w[l][0:F], pre_w[l][F:2 * F], pre_w[l][2 * F:3 * F]
        p12[l, 0:F, 0:F] = bf(P1)
        p12[l, 0:F, 128:128 + F] = bf(P2)
        q = (edge_tab @ enc_w[l] + enc_b[l]) @ P3 + pre_b[l]   # [10, F]
        qtabs.append(q.astype(np.float32))
        Wh = post_w[l][0:F]
        blocks = [post_w[l][(1 + i) * F:(2 + i) * F] for i in range(12)]
        s_plain = blocks[0] + blocks[1] + blocks[2]
        s_amp = blocks[4] + blocks[5] + blocks[6]
        s_att = blocks[8] + blocks[9] + blocks[10]
        lw = np.asarray(inputs["lin_w"], np.float32)[l]
        ordered = (blocks[0:4] + [s_plain, Wh] +
                   blocks[4:8] + [s_amp] +
                   blocks[8:12] + [s_att])
        for b, Wb in enumerate(ordered):
            wblk[l, 0:F, b * FP:b * FP + F] = bf(Wb @ lw)

    # per-layer edge-bias table in slot layout: qoh[l][:, s] = Q_l[attr[s]]
    # (zero for padded-with-no-edge / fake slots). Shared slot plan but
    # per-core attrs -> stored per core.
    for pc in per_core:
        oh_col = pc.pop("oh_col")
        valid = oh_col >= 0
        qoh = np.zeros((4, FP, TOT), np.float16)
        for l in range(4):
            qT = np.zeros((FP, 10), np.float16)
            qT[0:F] = bf(qtabs[l].T)
            qoh[l][:, valid] = qT[:, oh_col[valid]]
        pc["qoh"] = qoh

    ident = np.zeros((FP, FP), np.float16)
    np.fill_diagonal(ident, 1.0)

    mw = np.zeros((FP, 176), np.float32)
    mw[0:F, 0:100] = np.asarray(inputs["mlp_w1"], np.float32)
    mw[0:100, 100:150] = np.asarray(inputs["mlp_w2"], np.float32)
    mw[0:50, 150:175] = np.asarray(inputs["mlp_w3"], np.float32)
    mw[0:25, 175:176] = np.asarray(inputs["mlp_w4"], np.float32)

    bcol = np.zeros((FP, 32), np.float32)
    bcol[0:F, 0] = np.asarray(inputs["emb1_b"], np.float32)
    for l in range(4):
        lwl = np.asarray(inputs["lin_w"], np.float32)[l]
        pb = np.asarray(inputs["post_b"], np.float32)[l]
        lb = np.asarray(inputs["lin_b"], np.float32)[l]
        bcol[0:F, 5 + l] = lwl.T @ pb + lb
        bcol[0:F, 9 + l] = np.asarray(inputs["bn_g"], np.float32)[l]
        bcol[0:F, 13 + l] = np.asarray(inputs["bn_b"], np.float32)[l]
    bcol[0:100, 17] = np.asarray(inputs["mlp_b1"], np.float32)
    bcol[0:50, 18] = np.asarray(inputs["mlp_b2"], np.float32)
    bcol[0:25, 19] = np.asarray(inputs["mlp_b3"], np.float32)
    bcol[0:1, 20] = np.asarray(inputs["mlp_b4"], np.float32)
    bcol[:, 21] = EPS

    shared = dict(embw=embw, p12=p12, wblk=wblk,
                  ident=ident, mw=mw, bcol=bcol)
    meta = dict(W=W, TOT=TOT, n_fake=n_fake, windows=windows, boff=boff)
    return meta, shared, per_core


def _build(meta):
    W, TOT = meta["W"], meta["TOT"]
    windows = meta["windows"]
    boff = meta["boff"]
    n_fake = meta["n_fake"]

    nc = bacc.Bacc("TRN2", target_bir_lowering=False, debug=False,
                   num_devices=NCORES)

    def inp(name, shape, dt):
        return nc.dram_tensor(name, shape, dt, kind="ExternalInput").ap()

    xT_in = inp("xT", [8, W], HF)
    idx_in = inp("idx", [128, TOT // 16], I16)
    qoh_in = inp("qoh", [4, FP, TOT], HF)
    bcw_in = inp("bcw", [128, 5 * W], HF)
    sg_in = inp("sg", [W, GC], HF)
    embw_in = inp("embw", [8, FP], HF)
    p12_in = inp("p12", [4, FP, 256], HF)
    wblk_in = inp("wblk", [4, FP, 16 * FP], HF)
    ident_in = inp("ident", [FP, FP], HF)
    mw_in = inp("mw", [FP, 176], F32)
    bcol_in = inp("bcol", [FP, 32], F32)
    out_ext = nc.dram_tensor("out", [1, GC], F32, kind="ExternalOutput").ap()

    AF = mybir.ActivationFunctionType
    AL = mybir.AluOpType

    with tile.TileContext(nc) as tc:
        with (
            tc.tile_pool(name="res", bufs=1) as res,
            tc.tile_pool(name="wl", bufs=1) as wl,
            tc.tile_pool(name="gatp", bufs=2) as gatp,
            tc.tile_pool(name="sq1", bufs=1) as sq1,
            tc.tile_pool(name="qst", bufs=1) as qst,
            tc.tile_pool(name="bst", bufs=1) as bst,
            tc.tile_pool(name="tree", bufs=1) as treep,
            tc.tile_pool(name="stat1", bufs=1) as stat1,
            tc.tile_pool(name="stat2", bufs=2) as stat2,
            tc.tile_pool(name="dram", bufs=1, space="DRAM") as dram,
        ):
            hT = res.tile([FP, W], HF, tag="hT")
            hp2 = res.tile([FP, W + FP], HF, tag="hp2")
            preBN = res.tile([FP, W], HF, tag="preBN")
            idx_sb = res.tile([128, TOT // 16], I16, tag="idx")
            ident_sb = res.tile([FP, FP], HF, tag="ident")
            bcol_sb = res.tile([FP, 32], F32, tag="bcol")
            scr = res.tile([FP, 192], F32, tag="scr")

            nc.sync.dma_start(idx_sb[:], idx_in)
            nc.sync.dma_start(ident_sb[:], ident_in)
            nc.sync.dma_start(bcol_sb[:], bcol_in)
            nc.vector.memset(hp2[:, W:W + FP], 0.0)

            with (
                tc.tile_pool(name="psA", bufs=1, space="PSUM") as psA,
                tc.tile_pool(name="psG", bufs=2, space="PSUM") as psG,
            ):
                # ---- h0 = x @ emb1_w + emb1_b ----
                with tc.tile_pool(name="x0", bufs=2) as x0p:
                    embw_sb = wl.tile([8, FP], HF, tag="embw")
                    nc.sync.dma_start(embw_sb[:], embw_in)
                    for s in range(0, W, 512):
                        sw = min(512, W - s)
                        xc = x0p.tile([8, 512], HF, tag="xc")
                        nc.sync.dma_start(xc[:, 0:sw], xT_in[:, s:s + sw])
                        p0 = psA.tile([FP, 512], F32, tag="pH2")
                        nc.tensor.matmul(p0[:, 0:sw], embw_sb[:],
                                         xc[:, 0:sw], start=True, stop=True)
                        nc.scalar.activation(hT[:, s:s + sw], p0[:, 0:sw],
                                             AF.Identity, bias=bcol_sb[:, 0:1])

                for l in range(4):
                    p12_sb = wl.tile([FP, 256], HF, tag="p12")
                    wblk_sb = wl.tile([FP, 16 * FP], HF, tag="wblk")
                    nc.sync.dma_start(p12_sb[:], p12_in[l])
                    nc.sync.dma_start(wblk_sb[:], wblk_in[l])

                    # ---- Hp2 tokens ----
                    for s in range(0, W, 512):
                        sw = min(512, W - s)
                        p0 = psA.tile([FP, 512], F32, tag="pH2")
                        for k in range(0, sw, 128):
                            nc.tensor.matmul(
                                p0[:, k:k + 128], hT[:, s + k:s + k + 128],
                                p12_sb[:, 128:256], start=True, stop=True)
                        nc.scalar.activation(hp2[:, s:s + sw], p0[:, 0:sw], AF.Copy)

                    nslc = 0
                    for (D, noff, nw, soff) in windows:
                        slots = D * nw
                        gat = gatp.tile([128, 1, SLOT_CAP], HF, tag="gat")
                        nc.gpsimd.dma_gather(
                            gat[:, :, 0:slots], hp2[:],
                            idx_sb[:, soff // 16:(soff + slots) // 16],
                            slots, slots, elem_size=FP, transpose=True,
                            single_packet=False,
                            sbuf_tokens_per_rank=128, sbuf_free_dim_per_rank=256)
                        qoh_sb = qst.tile([128, SLOT_CAP], HF, tag="qoh")
                        nc.sync.dma_start(qoh_sb[:, 0:slots],
                                          qoh_in[l][:, soff:soff + slots])
                        bo = boff[(noff, nw)]
                        bc = bst.tile([128, 5 * NW_CAP], HF, tag="bc")
                        nc.sync.dma_start(bc[:, 0:5 * nw], bcw_in[:, bo:bo + 5 * nw])
                        amp_c = bc[:, 0 * nw:1 * nw]
                        att_c = bc[:, 1 * nw:2 * nw]
                        inv_c = bc[:, 2 * nw:3 * nw]
                        cpad_c = bc[:, 3 * nw:4 * nw]
                        msk_c = bc[:, 4 * nw:5 * nw]

                        # m = gathered + Q[attr] (in place); msq = m^2
                        nc.vector.tensor_tensor(gat[:, 0, 0:slots],
                                                gat[:, 0, 0:slots],
                                                qoh_sb[:, 0:slots], AL.add)
                        msq = sq1.tile([128, SLOT_CAP], HF, tag="msq")
                        nc.scalar.activation(msq[:, 0:slots], gat[:, 0, 0:slots],
                                             AF.Square)

                        # halving trees (vector) over slot-major blocks
                        def tree(op, leaf3d, buf, final):
                            """final: (tile, off) dst for the last level; the
                            leaf buffer ref is returned as-is when D == 1."""
                            def sl(t, is3, a, b):
                                return t[:, 0, a:b] if is3 else t[:, a:b]
                            if D == 1:
                                return (buf, leaf3d, 0)
                            width = slots
                            cur, cur3, cur_off = buf, leaf3d, 0
                            use_a = True
                            while width > nw:
                                half = width // 2
                                if half == nw and final is not None:
                                    dstt, dsto = final
                                else:
                                    dstt = treep.tile([128, SLOT_CAP // 2], HF,
                                                      tag=("tpA" if use_a
                                                           else "tpB"))
                                    dsto = 0
                                nc.vector.tensor_tensor(
                                    dstt[:, dsto:dsto + half],
                                    sl(cur, cur3, cur_off, cur_off + half),
                                    sl(cur, cur3, cur_off + half,
                                       cur_off + width), op)
                                cur, cur3, cur_off, width = dstt, False, dsto, half
                                use_a = not use_a
                            return (cur, False, cur_off)

                        mn_s = stat2.tile([128, NW_CAP], HF, tag="mn")
                        mx_s = stat2.tile([128, NW_CAP], HF, tag="mx")
                        sm_s = stat1.tile([128, NW_CAP], HF, tag="sm")
                        sq_s = stat1.tile([128, NW_CAP], HF, tag="sq")
                        mn_t, mn_3, mn_o = tree(AL.min, True, gat, (mn_s, 0))
                        mx_t, mx_3, mx_o = tree(AL.max, True, gat, (mx_s, 0))
                        sm_t, sm_3, sm_o = tree(AL.add, True, gat, (sm_s, 0))
                        sq_t, sq_3, sq_o = tree(AL.add, False, msq, (sq_s, 0))

                        # pad-replication correction: sum -= cpad * slot0
                        if D > 1:
                            t0 = stat1.tile([128, NW_CAP], HF, tag="c0")
                            nc.vector.tensor_tensor(t0[:, 0:nw],
                                                    gat[:, 0, 0:nw],
                                                    cpad_c, AL.mult)
                            nc.vector.tensor_tensor(sm_s[:, 0:nw], sm_s[:, 0:nw],
                                                    t0[:, 0:nw], AL.subtract)
                            nc.vector.tensor_tensor(t0[:, 0:nw], msq[:, 0:nw],
                                                    cpad_c, AL.mult)
                            nc.vector.tensor_tensor(sq_s[:, 0:nw], sq_s[:, 0:nw],
                                                    t0[:, 0:nw], AL.subtract)

                        def ref_sl(r, a, b):
                            t, is3, o = r
                            return t[:, 0, o + a:o + b] if is3 else t[:, o + a:o + b]

                        # mean / std
                        mean_s = stat2.tile([128, NW_CAP], HF, tag="mean")
                        nc.vector.tensor_tensor(mean_s[:, 0:nw],
                                                ref_sl((sm_t, sm_3, sm_o), 0, nw),
                                                inv_c, AL.mult)
                        var_s = stat1.tile([128, NW_CAP], HF, tag="var")
                        nc.vector.tensor_tensor(var_s[:, 0:nw],
                                                ref_sl((sq_t, sq_3, sq_o), 0, nw),
                                                inv_c, AL.mult)
                        t1 = stat1.tile([128, NW_CAP], HF, tag="c1")
                        nc.vector.tensor_tensor(t1[:, 0:nw], mean_s[:, 0:nw],
                                                mean_s[:, 0:nw], AL.mult)
                        nc.vector.tensor_tensor(var_s[:, 0:nw], var_s[:, 0:nw],
                                                t1[:, 0:nw], AL.subtract)
                        nc.vector.tensor_scalar_max(var_s[:, 0:nw],
                                                    var_s[:, 0:nw], 0.0)
                        std_s = stat2.tile([128, NW_CAP], HF, tag="std")
                        nc.scalar.activation(std_s[:, 0:nw], var_s[:, 0:nw],
                                             AF.Sqrt, bias=bcol_sb[:, 21:22])

                        # post/lin in three PSUM groups + per-node scalers
                        for so in range(0, nw, 512):
                            sw = min(512, nw - so)
                            ph1 = psA.tile([FP, 512], F32, tag="pH1")
                            nc.tensor.matmul(ph1[:, 0:sw], p12_sb[:, 0:128],
                                             hT[:, noff + so:noff + so + sw],
                                             start=True, stop=True)
                            hp1m = stat2.tile([128, 512], HF, tag="hp1m")
                            nc.vector.tensor_tensor(hp1m[:, 0:sw], ph1[:, 0:sw],
                                                    msk_c[:, so:so + sw], AL.mult)

                            stats = [(mean_s, False, 0), (mn_t, mn_3, mn_o),
                                     (mx_t, mx_3, mx_o), (std_s, False, 0)]
                            pP = psG.tile([FP, 512], F32, tag="pP")
                            pA2 = psG.tile([FP, 512], F32, tag="pA2")
                            pT2 = psG.tile([FP, 512], F32, tag="pT2")
                            plan = (
                                [(pP, 5, (hT, False, noff))] +
                                [(pP, b, stats[b]) for b in range(4)] +
                                [(pP, 4, (hp1m, False, -so))] +
                                [(pA2, 6 + b, stats[b]) for b in range(4)] +
                                [(pA2, 10, (hp1m, False, -so))] +
                                [(pT2, 11 + b, stats[b]) for b in range(4)] +
                                [(pT2, 15, (hp1m, False, -so))]
                            )
                            cnt = {id(pP): 0, id(pA2): 0, id(pT2): 0}
                            for (bank, b, ref) in plan:
                                k = cnt[id(bank)]
                                nc.tensor.matmul(
                                    bank[:, 0:sw],
                                    wblk_sb[:, b * FP:(b + 1) * FP],
                                    ref_sl(ref, so, so + sw),
                                    start=(k == 0), stop=(k == 5 if bank is pP
                                                          else k == 4))
                                cnt[id(bank)] += 1

                            c1 = stat1.tile([128, 512], HF, tag="cb1")
                            nc.vector.tensor_tensor(c1[:, 0:sw], pA2[:, 0:sw],
                                                    amp_c[:, so:so + sw], AL.mult)
                            c2 = stat1.tile([128, 512], HF, tag="cb2")
                            nc.vector.tensor_tensor(c2[:, 0:sw], pT2[:, 0:sw],
                                                    att_c[:, so:so + sw], AL.mult)
                            v = stat2.tile([128, 512], F32, tag="vsum")
                            nc.vector.tensor_tensor(v[:, 0:sw], pP[:, 0:sw],
                                                    c1[:, 0:sw], AL.add)
                            nc.vector.tensor_tensor(v[:, 0:sw], v[:, 0:sw],
                                                    c2[:, 0:sw], AL.add)
                            nc.scalar.activation(
                                preBN[:, noff + so:noff + so + sw], v[:, 0:sw],
                                AF.Identity, bias=bcol_sb[:, 5 + l:6 + l],
                                accum_out=scr[:, nslc:nslc + 1])
                            sqt = stat1.tile([128, 512], HF, tag="sqt")
                            nc.scalar.activation(
                                sqt[:, 0:sw],
                                preBN[:, noff + so:noff + so + sw],
                                AF.Square, accum_out=scr[:, 64 + nslc:65 + nslc])
                            nslc += 1

                    # ---- BN (global over cores) ----
                    assert nslc <= 64
                    nc.vector.tensor_reduce(scr[:, 128:129], scr[:, 0:nslc],
                                            mybir.AxisListType.X, AL.add)
                    nc.vector.tensor_reduce(scr[:, 129:130],
                                            scr[:, 64:64 + nslc],
                                            mybir.AxisListType.X, AL.add)
                    ufake = preBN[:, W - 1:W]
                    nc.vector.tensor_scalar(out=scr[:, 130:131], in0=ufake,
                                            scalar1=float(n_fake), scalar2=None,
                                            op0=AL.mult)
                    nc.scalar.activation(scr[:, 131:132], ufake, AF.Square)
                    nc.vector.tensor_scalar(out=scr[:, 131:132],
                                            in0=scr[:, 131:132],
                                            scalar1=float(n_fake), scalar2=None,
                                            op0=AL.mult)
                    nc.vector.tensor_tensor(scr[:, 132:133], scr[:, 128:129],
                                            scr[:, 130:131], AL.subtract)
                    nc.vector.tensor_tensor(scr[:, 133:134], scr[:, 129:130],
                                            scr[:, 131:132], AL.subtract)

                    cc_in = dram.tile([FP, 2], F32, tag=f"ccin{l}")
                    cc_out = dram.tile([FP, 2], F32, tag=f"ccout{l}")
                    nc.gpsimd.dma_start(cc_in[:], scr[:, 132:134])
                    nc.gpsimd.collective_compute(
                        "AllReduce", AL.add,
                        replica_groups=[list(range(NCORES))],
                        ins=[cc_in.opt()], outs=[cc_out.opt()])
                    nc.sync.dma_start(scr[:, 134:136], cc_out[:])

                    nc.vector.tensor_scalar_mul(scr[:, 136:137],
                                                scr[:, 134:135], 1.0 / NTOT)
                    nc.vector.tensor_scalar_mul(scr[:, 137:138],
                                                scr[:, 135:136], 1.0 / NTOT)
                    nc.vector.tensor_tensor(scr[:, 138:139], scr[:, 136:137],
                                            scr[:, 136:137], AL.mult)
                    nc.vector.tensor_tensor(scr[:, 139:140], scr[:, 137:138],
                                            scr[:, 138:139], AL.subtract)
                    nc.scalar.activation(scr[:, 140:141], scr[:, 139:140],
                                         AF.Sqrt, bias=bcol_sb[:, 21:22])
                    nc.vector.reciprocal(scr[:, 141:142], scr[:, 140:141])
                    nc.vector.tensor_tensor(scr[:, 142:143], scr[:, 141:142],
                                            bcol_sb[:, 9 + l:10 + l], AL.mult)
                    nc.vector.tensor_tensor(scr[:, 143:144], scr[:, 136:137],
                                            scr[:, 142:143], AL.mult)
                    nc.vector.tensor_tensor(scr[:, 144:145],
                                            bcol_sb[:, 13 + l:14 + l],
                                            scr[:, 143:144], AL.subtract)

                    for s in range(0, W, 512):
                        sw = min(512, W - s)
                        nc.scalar.activation(hT[:, s:s + sw], preBN[:, s:s + sw],
                                             AF.Relu, bias=scr[:, 144:145],
                                             scale=scr[:, 142:143])

            # ---- pooling + MLP ----
            with tc.tile_pool(name="ps2", bufs=2, space="PSUM") as ps2:
                with tc.tile_pool(name="poolw", bufs=2) as poolw:
                    pooled_ps = ps2.tile([FP, GC], F32, tag="pool")
                    nchunks = W // 128
                    for t in range(nchunks):
                        tp = ps2.tile([FP, FP], HF, tag="ptr")
                        nc.tensor.transpose(tp[:], hT[:, t * 128:(t + 1) * 128],
                                            ident_sb[:])
                        hnm = poolw.tile([128, 128], HF, tag="hnm")
                        nc.scalar.activation(hnm[:], tp[:], AF.Copy)
                        sg_sb = poolw.tile([128, GC], HF, tag="sg")
                        nc.sync.dma_start(sg_sb[:], sg_in[t * 128:(t + 1) * 128])
                        nc.tensor.matmul(pooled_ps[:], hnm[:], sg_sb[:],
                                         start=(t == 0), stop=(t == nchunks - 1))
                    pooled = res.tile([FP, GC], F32, tag="pooled")
                    nc.scalar.activation(pooled[:], pooled_ps[:], AF.Copy)

                mw_sb = wl.tile([FP, 176], F32, tag="mw")
                nc.sync.dma_start(mw_sb[:], mw_in)
                zp = ps2.tile([FP, GC], F32, tag="zp")
                z1 = res.tile([FP, GC], F32, tag="z1")
                nc.tensor.matmul(zp[0:100, :], mw_sb[0:128, 0:100], pooled[:],
                                 start=True, stop=True)
                nc.scalar.activation(z1[0:100, :], zp[0:100, :], AF.Relu,
                                     bias=bcol_sb[0:100, 17:18])
                zp2 = ps2.tile([FP, GC], F32, tag="zp")
                z2 = res.tile([FP, GC], F32, tag="z2")
                nc.tensor.matmul(zp2[0:50, :], mw_sb[0:100, 100:150],
                                 z1[0:100, :], start=True, stop=True)
                nc.scalar.activation(z2[0:50, :], zp2[0:50, :], AF.Relu,
                                     bias=bcol_sb[0:50, 18:19])
                zp3 = ps2.tile([FP, GC], F32, tag="zp")
                z3 = res.tile([FP, GC], F32, tag="z3")
                nc.tensor.matmul(zp3[0:25, :], mw_sb[0:50, 150:175],
                                 z2[0:50, :], start=True, stop=True)
                nc.scalar.activation(z3[0:25, :], zp3[0:25, :], AF.Relu,
                                     bias=bcol_sb[0:25, 19:20])
                zp4 = ps2.tile([FP, GC], F32, tag="zp")
                z4 = res.tile([1, GC], F32, tag="z4")
                nc.tensor.matmul(zp4[0:1, :], mw_sb[0:25, 175:176],
                                 z3[0:25, :], start=True, stop=True)
                nc.scalar.activation(z4[:], zp4[0:1, :], AF.Identity,
                                     bias=bcol_sb[0:1, 20:21])
                nc.sync.dma_start(out_ext, z4[:])

    nc.compile()
    return nc


def _execute(nc, in_maps, n_timing=3):
    """Run the compiled program via PJRT (axon); returns (results, best_ns)."""
    import jax
    from jax.sharding import Mesh, PartitionSpec, NamedSharding
    from jax.experimental.shard_map import shard_map

    bass2jax.install_neuronx_cc_hook()
    partition_name = (nc.partition_id_tensor.name
                      if nc.partition_id_tensor else None)

    in_names, out_names, out_avals, zero_outs = [], [], [], []
    for alloc in nc.m.functions[0].allocations:
        if not isinstance(alloc, mybir.MemoryLocationSet):
            continue
        name = alloc.memorylocations[0].name
        if alloc.kind == "ExternalInput":
            if name != partition_name:
                in_names.append(name)
        elif alloc.kind == "ExternalOutput":
            out_names.append(name)
            shape = tuple(alloc.tensor_shape)
            dtype = mybir.dt.np(alloc.dtype)
            out_avals.append(jax.core.ShapedArray(shape, dtype))
            zero_outs.append(np.zeros(shape, dtype))
    n_params = len(in_names)
    n_outs = len(out_avals)
    all_names = in_names + out_names + ([partition_name] if partition_name else [])

    def _body(*args):
        operands = list(args)
        if partition_name is not None:
            operands.append(bass2jax.partition_id_tensor())
        outs = bass2jax._bass_exec_p.bind(
            *operands,
            out_avals=tuple(out_avals),
            in_names=tuple(all_names),
            out_names=tuple(out_names),
            lowering_input_output_aliases=(),
            sim_require_finite=False,
            sim_require_nnan=False,
            nc=nc,
        )
        return tuple(outs)

    devices = jax.devices()[:NCORES]
    mesh = Mesh(np.asarray(devices), ("core",))
    in_specs = (PartitionSpec("core"),) * (n_params + n_outs)
    out_specs = (PartitionSpec("core"),) * n_outs
    donate = tuple(range(n_params, n_params + n_outs))
    sharded = jax.jit(
        shard_map(_body, mesh=mesh, in_specs=in_specs, out_specs=out_specs,
                  check_rep=False),
        donate_argnums=donate, keep_unused=True)

    # Pre-shard inputs onto the mesh: without an explicit NamedSharding the
    # jit inserts a per-argument reshard executable on EVERY call.
    shard = NamedSharding(mesh, PartitionSpec("core"))
    concat_in = [
        np.concatenate([np.asarray(in_maps[c][nm]) for c in range(NCORES)],
                       axis=0)
        for nm in in_names
    ]
    concat_in = [jax.device_put(a, shard) for a in concat_in]
    jax.block_until_ready(concat_in)

    def make_zs():
        return [jax.device_put(
                    np.zeros((NCORES * z.shape[0], *z.shape[1:]), z.dtype),
                    shard)
                for z in zero_outs]

    def one_call(zs):
        outs = sharded(*concat_in, *zs)
        jax.block_until_ready(outs)
        return outs

    out_arrs = one_call(make_zs())  # warmup / compile
    zs_sets = [make_zs() for _ in range(n_timing)]
    for zs in zs_sets:
        jax.block_until_ready(zs)
    best = None
    for zs in zs_sets:
        t0 = time.perf_counter()
        one_call(zs)
        dt = time.perf_counter() - t0
        best = dt if best is None else min(best, dt)
    results = [
        {nm: np.asarray(out_arrs[i]).reshape(NCORES, *out_avals[i].shape)[c]
         for i, nm in enumerate(out_names)}
        for c in range(NCORES)
    ]
    return results, int((best or 0) * 1e9)


def kernel(**inputs):
    global LAST_HW_EXEC_NS
    meta, shared, per_core = _prep(inputs)
    nc = _build(meta)
    in_maps = []
    for c in range(NCORES):
        pc = per_core[c]
        in_maps.append(dict(
            xT=pc["xT"], idx=pc["idx_tab"], qoh=pc["qoh"], bcw=pc["bcw"],
            sg=pc["sg"], embw=shared["embw"], p12=shared["p12"],
            wblk=shared["wblk"],
            ident=shared["ident"], mw=shared["mw"], bcol=shared["bcol"]))
    results, ns = _execute(nc, in_maps)
    LAST_HW_EXEC_NS = ns
    out = np.concatenate([results[c]["out"][0] for c in range(NCORES)])
    return out.reshape(G, 1).astype(np.float32)


# revision 18
# speedup vs baseline: 1.5474x; 1.1330x over previous
"""Trainium2 Bass kernel for the 4-layer PNA GNN (nn_Net_70025146794268).

Self-contained: accepts FULL inputs, shards graph-parallel over 8 NeuronCores,
runs a single SPMD Bass/Tile program, gathers the [G,1] output on host.

Per-core design (128 graphs, 16128 nodes, 57600 edges):
  * feature-major layouts throughout: tensors are [128 feat-partitions, nodes]
  * message m = Hp2[src] + Q[edge_attr] (pre_b folded into Q); the dst-side
    term Hp1[dst] is folded into the post matmul as extra K-blocks because
    mean/min/max shift by it and std is shift-invariant
  * per-node degree-padded slot tables (D in {1,2,4,8,16} regions) gathered via
    SBUF-source transpose dma_gather from the Hp2 token buffer; all four PNA
    stats computed by halving trees over the slot-major padded layout
  * the Q[edge_attr] term is precomputed host-side per layer (qoh) and
    streamed from HBM, so the message add is one vector op per window
  * GpSimd runs ONLY the gathers (single ucode library, no MPC reloads);
    all tensor_tensor work lives on the Vector engine
  * post_nn/lin matmuls are regrouped into three PSUM groups (plain/amp/att);
    the per-node degree scalers multiply the group outputs, which removes the
    10 per-window broadcast multiplies the naive layout needs
  * BatchNorm is global over all 129024 nodes: per-core sums all-reduced via
    a tiny gpsimd collective each layer
  * pooling via per-graph 0/1 matmuls, then the readout MLP on-device in f32
"""

import time

import numpy as np
import ml_dtypes

import concourse.bacc as bacc
import concourse.tile as tile
import concourse.mybir as mybir
from concourse import bass2jax

HF = mybir.dt.float16
F32 = mybir.dt.float32
I16 = mybir.dt.int16

G, NPG, EPG, F = 1024, 126, 450, 126
NCORES = 8
GC = G // NCORES            # 128 graphs per core
NLOC = GC * NPG             # 16128 nodes per core
ELOC = GC * EPG             # 57600 edges per core
FP = 128                    # padded feature dim
EPS = 1e-5
NTOT = float(G * NPG)       # BN normalizer (all nodes, all cores)
SLOT_CAP = 3840             # max padded slots per window
NW_CAP = 1024               # max nodes per window

LAST_HW_EXEC_NS = None


def _np2(d):
    p = 1
    while p < d:
        p *= 2
    return p


def _wrap_idx_table(flat):
    """[n] int16 -> [128, n//16] wrapped (i%16, i//16), replicated x8."""
    n = len(flat)
    assert n % 16 == 0
    tab = np.zeros((128, n // 16), np.int16)
    a = np.asarray(flat, np.int16).reshape(n // 16, 16).T  # [16, n//16]
    for g in range(8):
        tab[16 * g:16 * (g + 1)] = a
    return tab


def _prep(inputs):
    """Host-side preprocessing: common layout plan + per-core tables."""
    x = np.asarray(inputs["x"], np.float32)
    ei = np.asarray(inputs["edge_index"], np.int64)
    ea = np.asarray(inputs["edge_attr"], np.int64)
    dh = np.asarray(inputs["deg_hist"], np.float64)

    bins = np.arange(dh.shape[0], dtype=np.float64)
    avg_log = float((dh * np.log(bins + 1.0)).sum() / dh.sum())

    cores = []
    for c in range(NCORES):
        n0, e0 = c * NLOC, c * ELOC
        cores.append(dict(
            src=ei[0, e0:e0 + ELOC] - n0,
            dst=ei[1, e0:e0 + ELOC] - n0,
            at=ea[e0:e0 + ELOC],
        ))
        cores[-1]["deg"] = np.bincount(cores[-1]["dst"], minlength=NLOC)

    dmax = max(int(co["deg"].max()) for co in cores)
    if dmax > 32:
        raise RuntimeError(f"max degree {dmax} > 32 unsupported")
    # Degree-bracket regions (descending capacity, deg<=1 last), 128-node
    # aligned. A region of capacity D holds nodes with deg <= D; slots past
    # a node's degree replicate edge 0 and are fixed by the cpad correction.
    # Region tails are filled by "borrowing" the highest remaining
    # lower-degree nodes; leftovers become fake nodes. The capacity set is
    # searched to minimize the total slot count (the GpSimd descriptor-
    # generation bottleneck).
    maxcnt = {d: 0 for d in range(dmax + 1)}
    for co in cores:
        dvals = np.maximum(co["deg"], 1)
        for d in range(1, dmax + 1):
            maxcnt[d] = max(maxcnt[d], int((dvals == d).sum()))

    core_dvals = [np.maximum(co["deg"], 1) for co in cores]

    def plan_regions(caps):
        """caps: descending capacities ending in 1 -> (npad, TOT)."""
        npad = {}
        tot = 0
        for i, cap in enumerate(caps):
            lo = caps[i + 1] if i + 1 < len(caps) else 0
            cnt = max(int(((dv > lo) & (dv <= cap)).sum())
                      for dv in core_dvals)
            n = -(-cnt // 128) * 128
            if cap == 1:
                n = -(-(cnt + 128) // 128) * 128  # >=128 fakes
            npad[cap] = n
            tot += n * cap
        return npad, tot

    best = None
    for thresh in (1, 32, 64, 96, 128, 192, 256, 384):
        caps = sorted({d for d in range(2, dmax + 1) if maxcnt[d] >= thresh}
                      | {dmax, 1}, reverse=True)
        npad_c, tot_c = plan_regions(caps)
        if best is None or tot_c < best[2]:
            best = (caps, npad_c, tot_c)
    REG_DS, npad, _ = best

    windows = []              # (d, node_off, nw, slot_off)
    node_off, slot_off = 0, 0
    base = {}
    for d in REG_DS:
        base[d] = node_off
        win = max(128, min(NW_CAP, (SLOT_CAP // d) // 128 * 128))
        assert win * d <= SLOT_CAP
        off = 0
        while off < npad[d]:
            nw = min(win, npad[d] - off)
            windows.append((d, node_off + off, nw, slot_off))
            slot_off += d * nw
            off += nw
        node_off += npad[d]
    W, TOT = node_off, slot_off
    n_fake = W - NLOC
    assert W % 128 == 0 and TOT % 128 == 0, (W, TOT)

    boff = {}
    o = 0
    for (D, noff, nw, soff) in windows:
        boff[(noff, nw)] = o
        o += 5 * nw

    per_core = []
    for c, co in enumerate(cores):
        src, dst, at, deg = co["src"], co["dst"], co["at"], co["deg"]
        # greedy descending fill: region d gets its exact-degree-d nodes,
        # then borrows the highest remaining lower-degree nodes to fill the
        # common 128-aligned region size (leftover positions become fakes)
        dvals = np.maximum(deg, 1)
        order = np.argsort(-dvals, kind="stable")
        posmap = np.zeros(NLOC, np.int64)
        ptr = 0
        for d in REG_DS:
            take = min(npad[d], NLOC - ptr)
            nodes = order[ptr:ptr + take]
            assert take == 0 or int(dvals[nodes].max()) <= d
            posmap[nodes] = base[d] + np.arange(take)
            ptr += take
        assert ptr == NLOC

        eorder = np.argsort(dst, kind="stable")
        s_sorted = src[eorder]
        a_sorted = at[eorder]
        estart = np.zeros(NLOC + 1, np.int64)
        np.cumsum(np.bincount(dst, minlength=NLOC), out=estart[1:])

        node_at = np.full(W, -1, np.int64)
        node_at[posmap] = np.arange(NLOC)

        tok = np.full(TOT, W, np.int64)      # default: zero token
        oh_col = np.full(TOT, -1, np.int64)
        cpadv = np.zeros(W, np.float32)
        for (D, noff, nw, soff) in windows:
            nodes = node_at[noff:noff + nw]
            for j in range(nw):
                n = nodes[j]
                if n < 0:
                    cpadv[noff + j] = D
                    continue
                d = int(deg[n])
                cpadv[noff + j] = D - d
                if d == 0:
                    continue
                e0 = estart[n]
                for k in range(D):
                    e = e0 + (k if k < d else 0)
                    s = soff + k * nw + j
                    tok[s] = posmap[s_sorted[e]]
                    oh_col[s] = a_sorted[e]

        dcl = np.maximum(deg.astype(np.float64), 1.0)
        ampv = np.zeros(W, np.float32)
        attv = np.zeros(W, np.float32)
        invv = np.ones(W, np.float32)
        mskv = np.zeros(W, np.float32)
        ampv[posmap] = (np.log(dcl + 1.0) / avg_log).astype(np.float32)
        attv[posmap] = (avg_log / np.log(dcl + 1.0)).astype(np.float32)
        invv[posmap] = (1.0 / dcl).astype(np.float32)
        mskv[posmap] = (deg > 0).astype(np.float32)

        bcw = np.zeros((128, 5 * W), np.float16)
        o = 0
        for (D, noff, nw, soff) in windows:
            blk = np.concatenate([
                ampv[noff:noff + nw], attv[noff:noff + nw],
                invv[noff:noff + nw], cpadv[noff:noff + nw],
                mskv[noff:noff + nw]])
            bcw[:, o:o + 5 * nw] = blk[None, :].astype(np.float16)
            o += 5 * nw

        xTf = np.zeros((8, W), np.float32)
        xTf[0:5, posmap] = x[c * NLOC:(c + 1) * NLOC].T
        xT = xTf.astype(np.float16)

        sg = np.zeros((W, GC), np.float16)
        gid = np.repeat(np.arange(GC), NPG)
        sg[posmap, gid] = 1.0

        per_core.append(dict(
            idx_tab=_wrap_idx_table(tok.astype(np.int16)),
            oh_col=oh_col, bcw=bcw, xT=xT, sg=sg))

    # ---- shared weights in device layouts ----
    def bf(a):
        return np.asarray(a, np.float32).astype(np.float16)

    emb1_w = np.asarray(inputs["emb1_w"], np.float32)
    pre_w = np.asarray(inputs["pre_w"], np.float32)
    post_w = np.asarray(inputs["post_w"], np.float32)
    edge_tab = np.asarray(inputs["edge_tab"], np.float32)
    enc_w = np.asarray(inputs["enc_w"], np.float32)
    enc_b = np.asarray(inputs["enc_b"], np.float32)
    pre_b = np.asarray(inputs["pre_b"], np.float32)

    embw = np.zeros((8, FP), np.float16)
    embw[0:5, 0:F] = bf(emb1_w)

    p12 = np.zeros((4, FP, 256), np.float16)
    qtabs = []
    # wblk block order (16 blocks of FP cols):
    #   plain group: [W0 W1 W2 W3] (mean,mn,mx,std), s_plain, Wh
    #   amp   group: [W4..W7], s_amp
    #   att   group: [W8..W11], s_att
    wblk = np.zeros((4, FP, 16 * FP), np.float16)
    for l in range(4):
        P1, P2, P3 = pre_w[l][0:F], pre_w[l][F:2 * F], pre_w[l][2 * F:3 * F]
        p12[l, 0:F, 0:F] = bf(P1)
        p12[l, 0:F, 128:128 + F] = bf(P2)
        q = (edge_tab @ enc_w[l] + enc_b[l]) @ P3 + pre_b[l]   # [10, F]
        qtabs.append(q.astype(np.float32))
        Wh = post_w[l][0:F]
        blocks = [post_w[l][(1 + i) * F:(2 + i) * F] for i in range(12)]
        s_plain = blocks[0] + blocks[1] + blocks[2]
        s_amp = blocks[4] + blocks[5] + blocks[6]
        s_att = blocks[8] + blocks[9] + blocks[10]
        lw = np.asarray(inputs["lin_w"], np.float32)[l]
        ordered = (blocks[0:4] + [s_plain, Wh] +
                   blocks[4:8] + [s_amp] +
                   blocks[8:12] + [s_att])
        for b, Wb in enumerate(ordered):
            wblk[l, 0:F, b * FP:b * FP + F] = bf(Wb @ lw)

    # per-layer edge-bias table in slot layout: qoh[l][:, s] = Q_l[attr[s]]
    # (zero for padded-with-no-edge / fake slots). Shared slot plan but
    # per-core attrs -> stored per core.
    for pc in per_core:
        oh_col = pc.pop("oh_col")
        valid = oh_col >= 0
        qoh = np.zeros((4, FP, TOT), np.float16)
        for l in range(4):
            qT = np.zeros((FP, 10), np.float16)
            qT[0:F] = bf(qtabs[l].T)
            qoh[l][:, valid] = qT[:, oh_col[valid]]
        pc["qoh"] = qoh

    ident = np.zeros((FP, FP), np.float16)
    np.fill_diagonal(ident, 1.0)

    mw = np.zeros((FP, 176), np.float32)
    mw[0:F, 0:100] = np.asarray(inputs["mlp_w1"], np.float32)
    mw[0:100, 100:150] = np.asarray(inputs["mlp_w2"], np.float32)
    mw[0:50, 150:175] = np.asarray(inputs["mlp_w3"], np.float32)
    mw[0:25, 175:176] = np.asarray(inputs["mlp_w4"], np.float32)

    bcol = np.zeros((FP, 32), np.float32)
    bcol[0:F, 0] = np.asarray(inputs["emb1_b"], np.float32)
    for l in range(4):
        lwl = np.asarray(inputs["lin_w"], np.float32)[l]
        pb = np.asarray(inputs["post_b"], np.float32)[l]
        lb = np.asarray(inputs["lin_b"], np.float32)[l]
        bcol[0:F, 5 + l] = lwl.T @ pb + lb
        bcol[0:F, 9 + l] = np.asarray(inputs["bn_g"], np.float32)[l]
        bcol[0:F, 13 + l] = np.asarray(inputs["bn_b"], np.float32)[l]
    bcol[0:100, 17] = np.asarray(inputs["mlp_b1"], np.float32)
    bcol[0:50, 18] = np.asarray(inputs["mlp_b2"], np.float32)
    bcol[0:25, 19] = np.asarray(inputs["mlp_b3"], np.float32)
    bcol[0:1, 20] = np.asarray(inputs["mlp_b4"], np.float32)
    bcol[:, 21] = EPS

    shared = dict(embw=embw, p12=p12, wblk=wblk,
                  ident=ident, mw=mw, bcol=bcol)
    meta = dict(W=W, TOT=TOT, n_fake=n_fake, windows=windows, boff=boff)
    return meta, shared, per_core


def _build(meta):
    W, TOT = meta["W"], meta["TOT"]
    windows = meta["windows"]
    boff = meta["boff"]
    n_fake = meta["n_fake"]

    nc = bacc.Bacc("TRN2", target_bir_lowering=False, debug=False,
                   num_devices=NCORES)

    def inp(name, shape, dt):
        return nc.dram_tensor(name, shape, dt, kind="ExternalInput").ap()

    xT_in = inp("xT", [8, W], HF)
    idx_in = inp("idx", [128, TOT // 16], I16)
    qoh_in = inp("qoh", [4, FP, TOT], HF)
    bcw_in = inp("bcw", [128, 5 * W], HF)
    sg_in = inp("sg", [W, GC], HF)
    embw_in = inp("embw", [8, FP], HF)
    p12_in = inp("p12", [4, FP, 256], HF)
    wblk_in = inp("wblk", [4, FP, 16 * FP], HF)
    ident_in = inp("ident", [FP, FP], HF)
    mw_in = inp("mw", [FP, 176], F32)
    bcol_in = inp("bcol", [FP, 32], F32)
    out_ext = nc.dram_tensor("out", [1, GC], F32, kind="ExternalOutput").ap()

    AF = mybir.ActivationFunctionType
    AL = mybir.AluOpType

    with tile.TileContext(nc) as tc:
        with (
            tc.tile_pool(name="res", bufs=1) as res,
            tc.tile_pool(name="wl", bufs=1) as wl,
            tc.tile_pool(name="gatp", bufs=2) as gatp,
            tc.tile_pool(name="sq1", bufs=1) as sq1,
            tc.tile_pool(name="qst", bufs=1) as qst,
            tc.tile_pool(name="bst", bufs=1) as bst,
            tc.tile_pool(name="tree", bufs=1) as treep,
            tc.tile_pool(name="stat1", bufs=1) as stat1,
            tc.tile_pool(name="stat2", bufs=2) as stat2,
            tc.tile_pool(name="dram", bufs=1, space="DRAM") as dram,
        ):
            hT = res.tile([FP, W], HF, tag="hT")
            hp2 = res.tile([FP, W + FP], HF, tag="hp2")
            preBN = res.tile([FP, W], HF, tag="preBN")
            idx_sb = res.tile([128, TOT // 16], I16, tag="idx")
            ident_sb = res.tile([FP, FP], HF, tag="ident")
            bcol_sb = res.tile([FP, 32], F32, tag="bcol")
            scr = res.tile([FP, 192], F32, tag="scr")

            nc.sync.dma_start(idx_sb[:], idx_in)
            nc.sync.dma_start(ident_sb[:], ident_in)
            nc.sync.dma_start(bcol_sb[:], bcol_in)
            nc.vector.memset(hp2[:, W:W + FP], 0.0)

            with (
                tc.tile_pool(name="psA", bufs=1, space="PSUM") as psA,
                tc.tile_pool(name="psG", bufs=2, space="PSUM") as psG,
            ):
                # ---- h0 = x @ emb1_w + emb1_b ----
                with tc.tile_pool(name="x0", bufs=2) as x0p:
                    embw_sb = wl.tile([8, FP], HF, tag="embw")
                    nc.sync.dma_start(embw_sb[:], embw_in)
                    for s in range(0, W, 512):
                        sw = min(512, W - s)
                        xc = x0p.tile([8, 512], HF, tag="xc")
                        nc.sync.dma_start(xc[:, 0:sw], xT_in[:, s:s + sw])
                        p0 = psA.tile([FP, 512], F32, tag="pH2")
                        nc.tensor.matmul(p0[:, 0:sw], embw_sb[:],
                                         xc[:, 0:sw], start=True, stop=True)
                        nc.scalar.activation(hT[:, s:s + sw], p0[:, 0:sw],
                                             AF.Identity, bias=bcol_sb[:, 0:1])

                for l in range(4):
                    p12_sb = wl.tile([FP, 256], HF, tag="p12")
                    wblk_sb = wl.tile([FP, 16 * FP], HF, tag="wblk")
                    nc.sync.dma_start(p12_sb[:], p12_in[l])
                    nc.sync.dma_start(wblk_sb[:], wblk_in[l])

                    # ---- Hp2 tokens ----
                    for s in range(0, W, 512):
                        sw = min(512, W - s)
                        p0 = psA.tile([FP, 512], F32, tag="pH2")
                        for k in range(0, sw, 128):
                            nc.tensor.matmul(
                                p0[:, k:k + 128], hT[:, s + k:s + k + 128],
                                p12_sb[:, 128:256], start=True, stop=True)
                        nc.scalar.activation(hp2[:, s:s + sw], p0[:, 0:sw], AF.Copy)

                    nslc = 0
                    for (D, noff, nw, soff) in windows:
                        slots = D * nw
                        gat = gatp.tile([128, 1, SLOT_CAP], HF, tag="gat")
                        nc.gpsimd.dma_gather(
                            gat[:, :, 0:slots], hp2[:],
                            idx_sb[:, soff // 16:(soff + slots) // 16],
                            slots, slots, elem_size=FP, transpose=True,
                            single_packet=False,
                            sbuf_tokens_per_rank=128, sbuf_free_dim_per_rank=256)
                        qoh_sb = qst.tile([128, SLOT_CAP], HF, tag="qoh")
                        nc.sync.dma_start(qoh_sb[:, 0:slots],
                                          qoh_in[l][:, soff:soff + slots])
                        bo = boff[(noff, nw)]
                        bc = bst.tile([128, 5 * NW_CAP], HF, tag="bc")
                        nc.sync.dma_start(bc[:, 0:5 * nw], bcw_in[:, bo:bo + 5 * nw])
                        amp_c = bc[:, 0 * nw:1 * nw]
                        att_c = bc[:, 1 * nw:2 * nw]
                        inv_c = bc[:, 2 * nw:3 * nw]
                        cpad_c = bc[:, 3 * nw:4 * nw]
                        msk_c = bc[:, 4 * nw:5 * nw]

                        # m = gathered + Q[attr] (in place); msq = m^2
                        nc.vector.tensor_tensor(gat[:, 0, 0:slots],
                                                gat[:, 0, 0:slots],
                                                qoh_sb[:, 0:slots], AL.add)
                        msq = sq1.tile([128, SLOT_CAP], HF, tag="msq")
                        nc.scalar.activation(msq[:, 0:slots], gat[:, 0, 0:slots],
                                             AF.Square)

                        # halving trees (vector) over slot-major blocks;
                        # odd block counts fold their last block into block 0
                        def tree(op, leaf3d, buf, final):
                            """final: (tile, off) dst for the last level; the
                            leaf buffer ref is returned as-is when D == 1."""
                            def sl(t, is3, a, b):
                                return t[:, 0, a:b] if is3 else t[:, a:b]
                            if D == 1:
                                return (buf, leaf3d, 0)
                            mult = D
                            cur, cur3, cur_off = buf, leaf3d, 0
                            use_a = True
                            while mult > 1:
                                half = mult // 2
                                if half == 1 and final is not None:
                                    dstt, dsto = final
                                else:
                                    dstt = treep.tile([128, SLOT_CAP // 2], HF,
                                                      tag=("tpA" if use_a
                                                           else "tpB"))
                                    dsto = 0
                                nc.vector.tensor_tensor(
                                    dstt[:, dsto:dsto + half * nw],
                                    sl(cur, cur3, cur_off, cur_off + half * nw),
                                    sl(cur, cur3, cur_off + half * nw,
                                       cur_off + 2 * half * nw), op)
                                if mult % 2:
                                    nc.vector.tensor_tensor(
                                        dstt[:, dsto:dsto + nw],
                                        dstt[:, dsto:dsto + nw],
                                        sl(cur, cur3,
                                           cur_off + 2 * half * nw,
                                           cur_off + mult * nw), op)
                                cur, cur3, cur_off, mult = dstt, False, dsto, half
                                use_a = not use_a
                            return (cur, False, cur_off)

                        mn_s = stat2.tile([128, NW_CAP], HF, tag="mn")
                        mx_s = stat2.tile([128, NW_CAP], HF, tag="mx")
                        sm_s = stat1.tile([128, NW_CAP], HF, tag="sm")
                        sq_s = stat1.tile([128, NW_CAP], HF, tag="sq")
                        mn_t, mn_3, mn_o = tree(AL.min, True, gat, (mn_s, 0))
                        mx_t, mx_3, mx_o = tree(AL.max, True, gat, (mx_s, 0))
                        sm_t, sm_3, sm_o = tree(AL.add, True, gat, (sm_s, 0))
                        sq_t, sq_3, sq_o = tree(AL.add, False, msq, (sq_s, 0))

                        # pad-replication correction: sum -= cpad * slot0
                        if D > 1:
                            t0 = stat1.tile([128, NW_CAP], HF, tag="c0")
                            nc.vector.tensor_tensor(t0[:, 0:nw],
                                                    gat[:, 0, 0:nw],
                                                    cpad_c, AL.mult)
                            nc.vector.tensor_tensor(sm_s[:, 0:nw], sm_s[:, 0:nw],
                                                    t0[:, 0:nw], AL.subtract)
                            nc.vector.tensor_tensor(t0[:, 0:nw], msq[:, 0:nw],
                                                    cpad_c, AL.mult)
                            nc.vector.tensor_tensor(sq_s[:, 0:nw], sq_s[:, 0:nw],
                                                    t0[:, 0:nw], AL.subtract)

                        def ref_sl(r, a, b):
                            t, is3, o = r
                            return t[:, 0, o + a:o + b] if is3 else t[:, o + a:o + b]

                        # mean / std
                        mean_s = stat2.tile([128, NW_CAP], HF, tag="mean")
                        nc.vector.tensor_tensor(mean_s[:, 0:nw],
                                                ref_sl((sm_t, sm_3, sm_o), 0, nw),
                                                inv_c, AL.mult)
                        var_s = stat1.tile([128, NW_CAP], HF, tag="var")
                        nc.vector.tensor_tensor(var_s[:, 0:nw],
                                                ref_sl((sq_t, sq_3, sq_o), 0, nw),
                                                inv_c, AL.mult)
                        t1 = stat1.tile([128, NW_CAP], HF, tag="c1")
                        nc.vector.tensor_tensor(t1[:, 0:nw], mean_s[:, 0:nw],
                                                mean_s[:, 0:nw], AL.mult)
                        nc.vector.tensor_tensor(var_s[:, 0:nw], var_s[:, 0:nw],
                                                t1[:, 0:nw], AL.subtract)
                        # clip-to-0 via scalar Relu (DVE tensor_scalar is ~13x
                        # slower than tensor_tensor for this shape)
                        nc.scalar.activation(t1[:, 0:nw], var_s[:, 0:nw],
                                             AF.Relu)
                        std_s = stat2.tile([128, NW_CAP], HF, tag="std")
                        nc.scalar.activation(std_s[:, 0:nw], t1[:, 0:nw],
                                             AF.Sqrt, bias=bcol_sb[:, 21:22])

                        # post/lin in three PSUM groups + per-node scalers
                        for so in range(0, nw, 512):
                            sw = min(512, nw - so)
                            ph1 = psA.tile([FP, 512], F32, tag="pH1")
                            nc.tensor.matmul(ph1[:, 0:sw], p12_sb[:, 0:128],
                                             hT[:, noff + so:noff + so + sw],
                                             start=True, stop=True)
                            hp1m = stat2.tile([128, 512], HF, tag="hp1m")
                            nc.vector.tensor_tensor(hp1m[:, 0:sw], ph1[:, 0:sw],
                                                    msk_c[:, so:so + sw], AL.mult)

                            stats = [(mean_s, False, 0), (mn_t, mn_3, mn_o),
                                     (mx_t, mx_3, mx_o), (std_s, False, 0)]
                            pP = psG.tile([FP, 512], F32, tag="pP")
                            pA2 = psG.tile([FP, 512], F32, tag="pA2")
                            pT2 = psG.tile([FP, 512], F32, tag="pT2")
                            plan = (
                                [(pP, 5, (hT, False, noff))] +
                                [(pP, b, stats[b]) for b in range(4)] +
                                [(pP, 4, (hp1m, False, -so))] +
                                [(pA2, 6 + b, stats[b]) for b in range(4)] +
                                [(pA2, 10, (hp1m, False, -so))] +
                                [(pT2, 11 + b, stats[b]) for b in range(4)] +
                                [(pT2, 15, (hp1m, False, -so))]
                            )
                            cnt = {id(pP): 0, id(pA2): 0, id(pT2): 0}
                            for (bank, b, ref) in plan:
                                k = cnt[id(bank)]
                                nc.tensor.matmul(
                                    bank[:, 0:sw],
                                    wblk_sb[:, b * FP:(b + 1) * FP],
                                    ref_sl(ref, so, so + sw),
                                    start=(k == 0), stop=(k == 5 if bank is pP
                                                          else k == 4))
                                cnt[id(bank)] += 1

                            c1 = stat1.tile([128, 512], HF, tag="cb1")
                            nc.vector.tensor_tensor(c1[:, 0:sw], pA2[:, 0:sw],
                                                    amp_c[:, so:so + sw], AL.mult)
                            c2 = stat1.tile([128, 512], HF, tag="cb2")
                            nc.vector.tensor_tensor(c2[:, 0:sw], pT2[:, 0:sw],
                                                    att_c[:, so:so + sw], AL.mult)
                            v = stat2.tile([128, 512], F32, tag="vsum")
                            nc.vector.tensor_tensor(v[:, 0:sw], pP[:, 0:sw],
                                                    c1[:, 0:sw], AL.add)
                            nc.vector.tensor_tensor(v[:, 0:sw], v[:, 0:sw],
                                                    c2[:, 0:sw], AL.add)
                            nc.scalar.activation(
                                preBN[:, noff + so:noff + so + sw], v[:, 0:sw],
                                AF.Identity, bias=bcol_sb[:, 5 + l:6 + l],
                                accum_out=scr[:, nslc:nslc + 1])
                            sqt = stat1.tile([128, 512], HF, tag="sqt")
                            nc.scalar.activation(
                                sqt[:, 0:sw],
                                preBN[:, noff + so:noff + so + sw],
                                AF.Square, accum_out=scr[:, 64 + nslc:65 + nslc])
                            nslc += 1

                    # ---- BN (global over cores) ----
                    assert nslc <= 64
                    nc.vector.tensor_reduce(scr[:, 128:129], scr[:, 0:nslc],
                                            mybir.AxisListType.X, AL.add)
                    nc.vector.tensor_reduce(scr[:, 129:130],
                                            scr[:, 64:64 + nslc],
                                            mybir.AxisListType.X, AL.add)
                    ufake = preBN[:, W - 1:W]
                    nc.vector.tensor_scalar(out=scr[:, 130:131], in0=ufake,
                                            scalar1=float(n_fake), scalar2=None,
                                            op0=AL.mult)
                    nc.scalar.activation(scr[:, 131:132], ufake, AF.Square)
                    nc.vector.tensor_scalar(out=scr[:, 131:132],
                                            in0=scr[:, 131:132],
                                            scalar1=float(n_fake), scalar2=None,
                                            op0=AL.mult)
                    nc.vector.tensor_tensor(scr[:, 132:133], scr[:, 128:129],
                                            scr[:, 130:131], AL.subtract)
                    nc.vector.tensor_tensor(scr[:, 133:134], scr[:, 129:130],
                                            scr[:, 131:132], AL.subtract)

                    cc_in = dram.tile([FP, 2], F32, tag=f"ccin{l}")
                    cc_out = dram.tile([FP, 2], F32, tag=f"ccout{l}")
                    nc.gpsimd.dma_start(cc_in[:], scr[:, 132:134])
                    nc.gpsimd.collective_compute(
                        "AllReduce", AL.add,
                        replica_groups=[list(range(NCORES))],
                        ins=[cc_in.opt()], outs=[cc_out.opt()])
                    nc.sync.dma_start(scr[:, 134:136], cc_out[:])

                    nc.vector.tensor_scalar_mul(scr[:, 136:137],
                                                scr[:, 134:135], 1.0 / NTOT)
                    nc.vector.tensor_scalar_mul(scr[:, 137:138],
                                                scr[:, 135:136], 1.0 / NTOT)
                    nc.vector.tensor_tensor(scr[:, 138:139], scr[:, 136:137],
                                            scr[:, 136:137], AL.mult)
                    nc.vector.tensor_tensor(scr[:, 139:140], scr[:, 137:138],
                                            scr[:, 138:139], AL.subtract)
                    nc.scalar.activation(scr[:, 140:141], scr[:, 139:140],
                                         AF.Sqrt, bias=bcol_sb[:, 21:22])
                    nc.vector.reciprocal(scr[:, 141:142], scr[:, 140:141])
                    nc.vector.tensor_tensor(scr[:, 142:143], scr[:, 141:142],
                                            bcol_sb[:, 9 + l:10 + l], AL.mult)
                    nc.vector.tensor_tensor(scr[:, 143:144], scr[:, 136:137],
                                            scr[:, 142:143], AL.mult)
                    nc.vector.tensor_tensor(scr[:, 144:145],
                                            bcol_sb[:, 13 + l:14 + l],
                                            scr[:, 143:144], AL.subtract)

                    for s in range(0, W, 512):
                        sw = min(512, W - s)
                        nc.scalar.activation(hT[:, s:s + sw], preBN[:, s:s + sw],
                                             AF.Relu, bias=scr[:, 144:145],
                                             scale=scr[:, 142:143])

            # ---- pooling + MLP ----
            with tc.tile_pool(name="ps2", bufs=2, space="PSUM") as ps2:
                with tc.tile_pool(name="poolw", bufs=2) as poolw:
                    pooled_ps = ps2.tile([FP, GC], F32, tag="pool")
                    nchunks = W // 128
                    for t in range(nchunks):
                        tp = ps2.tile([FP, FP], HF, tag="ptr")
                        nc.tensor.transpose(tp[:], hT[:, t * 128:(t + 1) * 128],
                                            ident_sb[:])
                        hnm = poolw.tile([128, 128], HF, tag="hnm")
                        nc.scalar.activation(hnm[:], tp[:], AF.Copy)
                        sg_sb = poolw.tile([128, GC], HF, tag="sg")
                        nc.sync.dma_start(sg_sb[:], sg_in[t * 128:(t + 1) * 128])
                        nc.tensor.matmul(pooled_ps[:], hnm[:], sg_sb[:],
                                         start=(t == 0), stop=(t == nchunks - 1))
                    pooled = res.tile([FP, GC], F32, tag="pooled")
                    nc.scalar.activation(pooled[:], pooled_ps[:], AF.Copy)

                mw_sb = wl.tile([FP, 176], F32, tag="mw")
                nc.sync.dma_start(mw_sb[:], mw_in)
                zp = ps2.tile([FP, GC], F32, tag="zp")
                z1 = res.tile([FP, GC], F32, tag="z1")
                nc.tensor.matmul(zp[0:100, :], mw_sb[0:128, 0:100], pooled[:],
                                 start=True, stop=True)
                nc.scalar.activation(z1[0:100, :], zp[0:100, :], AF.Relu,
                                     bias=bcol_sb[0:100, 17:18])
                zp2 = ps2.tile([FP, GC], F32, tag="zp")
                z2 = res.tile([FP, GC], F32, tag="z2")
                nc.tensor.matmul(zp2[0:50, :], mw_sb[0:100, 100:150],
                                 z1[0:100, :], start=True, stop=True)
                nc.scalar.activation(z2[0:50, :], zp2[0:50, :], AF.Relu,
                                     bias=bcol_sb[0:50, 18:19])
                zp3 = ps2.tile([FP, GC], F32, tag="zp")
                z3 = res.tile([FP, GC], F32, tag="z3")
                nc.tensor.matmul(zp3[0:25, :], mw_sb[0:50, 150:175],
                                 z2[0:50, :], start=True, stop=True)
                nc.scalar.activation(z3[0:25, :], zp3[0:25, :], AF.Relu,
                                     bias=bcol_sb[0:25, 19:20])
                zp4 = ps2.tile([FP, GC], F32, tag="zp")
                z4 = res.tile([1, GC], F32, tag="z4")
                nc.tensor.matmul(zp4[0:1, :], mw_sb[0:25, 175:176],
                                 z3[0:25, :], start=True, stop=True)
                nc.scalar.activation(z4[:], zp4[0:1, :], AF.Identity,
                                     bias=bcol_sb[0:1, 20:21])
                nc.sync.dma_start(out_ext, z4[:])

    nc.compile()
    return nc


def _execute(nc, in_maps, n_timing=5):
    """Run the compiled program via PJRT (axon); returns (results, best_ns)."""
    import jax
    from jax.sharding import Mesh, PartitionSpec, NamedSharding
    from jax.experimental.shard_map import shard_map

    bass2jax.install_neuronx_cc_hook()
    partition_name = (nc.partition_id_tensor.name
                      if nc.partition_id_tensor else None)

    in_names, out_names, out_avals, zero_outs = [], [], [], []
    for alloc in nc.m.functions[0].allocations:
        if not isinstance(alloc, mybir.MemoryLocationSet):
            continue
        name = alloc.memorylocations[0].name
        if alloc.kind == "ExternalInput":
            if name != partition_name:
                in_names.append(name)
        elif alloc.kind == "ExternalOutput":
            out_names.append(name)
            shape = tuple(alloc.tensor_shape)
            dtype = mybir.dt.np(alloc.dtype)
            out_avals.append(jax.core.ShapedArray(shape, dtype))
            zero_outs.append(np.zeros(shape, dtype))
    n_params = len(in_names)
    n_outs = len(out_avals)
    all_names = in_names + out_names + ([partition_name] if partition_name else [])

    def _body(*args):
        operands = list(args)
        if partition_name is not None:
            operands.append(bass2jax.partition_id_tensor())
        outs = bass2jax._bass_exec_p.bind(
            *operands,
            out_avals=tuple(out_avals),
            in_names=tuple(all_names),
            out_names=tuple(out_names),
            lowering_input_output_aliases=(),
            sim_require_finite=False,
            sim_require_nnan=False,
            nc=nc,
        )
        return tuple(outs)

    devices = jax.devices()[:NCORES]
    mesh = Mesh(np.asarray(devices), ("core",))
    in_specs = (PartitionSpec("core"),) * (n_params + n_outs)
    out_specs = (PartitionSpec("core"),) * n_outs
    donate = tuple(range(n_params, n_params + n_outs))
    sharded = jax.jit(
        shard_map(_body, mesh=mesh, in_specs=in_specs, out_specs=out_specs,
                  check_rep=False),
        donate_argnums=donate, keep_unused=True)

    # Pre-shard inputs onto the mesh: without an explicit NamedSharding the
    # jit inserts a per-argument reshard executable on EVERY call.
    shard = NamedSharding(mesh, PartitionSpec("core"))
    concat_in = [
        np.concatenate([np.asarray(in_maps[c][nm]) for c in range(NCORES)],
                       axis=0)
        for nm in in_names
    ]
    concat_in = [jax.device_put(a, shard) for a in concat_in]
    jax.block_until_ready(concat_in)

    def make_zs():
        return [jax.device_put(
                    np.zeros((NCORES * z.shape[0], *z.shape[1:]), z.dtype),
                    shard)
                for z in zero_outs]

    def one_call(zs):
        outs = sharded(*concat_in, *zs)
        jax.block_until_ready(outs)
        return outs

    out_arrs = one_call(make_zs())  # warmup / compile
    zs_sets = [make_zs() for _ in range(n_timing)]
    for zs in zs_sets:
        jax.block_until_ready(zs)
    best = None
    for zs in zs_sets:
        t0 = time.perf_counter()
        one_call(zs)
        dt = time.perf_counter() - t0
        best = dt if best is None else min(best, dt)
    results = [
        {nm: np.asarray(out_arrs[i]).reshape(NCORES, *out_avals[i].shape)[c]
         for i, nm in enumerate(out_names)}
        for c in range(NCORES)
    ]
    return results, int((best or 0) * 1e9)


def kernel(**inputs):
    global LAST_HW_EXEC_NS
    meta, shared, per_core = _prep(inputs)
    nc = _build(meta)
    in_maps = []
    for c in range(NCORES):
        pc = per_core[c]
        in_maps.append(dict(
            xT=pc["xT"], idx=pc["idx_tab"], qoh=pc["qoh"], bcw=pc["bcw"],
            sg=pc["sg"], embw=shared["embw"], p12=shared["p12"],
            wblk=shared["wblk"],
            ident=shared["ident"], mw=shared["mw"], bcol=shared["bcol"]))
    results, ns = _execute(nc, in_maps)
    LAST_HW_EXEC_NS = ns
    out = np.concatenate([results[c]["out"][0] for c in range(NCORES)])
    return out.reshape(G, 1).astype(np.float32)
